# revision 26
# baseline (speedup 1.0000x reference)
"""Trainium2 Bass kernel for MultiHeadSelfAttention2D.

Problem: x(4,256,64,64); q,k,v,proj 1x1-conv projections; 4 heads x 64 dim;
full 4096x4096 attention per (batch,head); out = gamma*proj + x.

Sharding: 8 cores = batch(4) x query-half(2). Each core computes its full
output slice out[b][:, nhalf] on-device:
  - K,V projected from full x[b]; Q from its query half only.
  - Flash-style attention, entirely in the PE's (64,128) row-tiled mode so
    the array never mode-switches mid-loop and both 64-row groups stay busy:
      * S chunk: heads 2t and 2t+1 computed CONCURRENTLY (T0 rows 0-63,
        T8 rows 64-127) into the two banks of one [128,1024] PSUM tile.
      * exp: alternates between ScalarE (table exp) and DVE (Schraudolph
        int16 bit-trick, bitcast to bf16) so neither engine bottlenecks.
      * PV: keys split 64/64 across T0/T8, accumulated in separate PSUM
        banks (ops0/ops1), combined during normalize.
      * softmax denominator: ones-row appended to V^T (M=65); reciprocal
        via exp(-ln(x)) on ScalarE (same ACT table set as exp; no switch);
        broadcast across partitions with a K-padded ones matmul (in-mode).
  - Output projection is K=64 per head (also (64,128) mode), + residual.
Host only concatenates the 8 slices.
"""

import numpy as np

import concourse.bass as bass
import concourse.mybir as mybir
import concourse.tile as tile

B, C, H, W, HEADS = 4, 256, 64, 64, 4
HD = C // HEADS  # 64
HW = H * W  # 4096
NHALF = HW // 2  # 2048
NCHUNK = HW // 128  # 32 key chunks
QB = 512  # query block
NQB = NHALF // QB  # 4
SCALE = 1.0 / np.sqrt(HD)
F32 = mybir.dt.float32
BF16 = mybir.dt.bfloat16
I16 = mybir.dt.int16

LN2 = float(np.log(2.0))
SCH_A = float(SCALE) * 128.0 / LN2  # folds the 1/sqrt(hd) score scale
SCH_B = 127.0 * 128.0 - 7.42

# chunks the PV matmuls trail behind S/exp: must cover the exp latency
# (~0.7us) with PE chunk periods (~0.7us) so the in-order PE never waits
PV_LAG = 3


def _fix_tail_drain(nc, keep=1):
    """This walrus build rejects instructions with more than a couple of
    semaphore waits. Inserting a same-engine NoOp immediately before an
    instruction is semantically identical (the engine blocks at the NoOp
    instead), so split any excess waits onto adjacent NoOps."""
    fn = nc.m.functions[0]
    for bi, blk in enumerate(fn.blocks):
        insts = list(blk.instructions)
        changed = False
        new_list = []
        for ins in insts:
            si = ins.sync_info
            if si is not None and len(si.on_wait) > keep:
                waits = list(si.on_wait)
                kept, excess = waits[:keep], waits[keep:]
                for j, w in enumerate(excess):
                    new_list.append(
                        mybir.InstNoOp(
                            name=f"waitfix-{bi}-{ins.name}-{j}",
                            engine=ins.engine,
                            sync_info=mybir.SyncInfo(on_wait=[w], on_update=[]),
                        )
                    )
                ins.sync_info = mybir.SyncInfo(on_wait=kept, on_update=si.on_update)
                changed = True
            new_list.append(ins)
        if changed:
            blk.instructions = new_list


def build(fix=True):
    from concourse.masks import make_identity

    nc = bass.Bass("TRN2", target_bir_lowering=False)

    x_d = nc.dram_tensor("x", [C, HW], F32, kind="ExternalInput")
    xq_d = nc.dram_tensor("xq", [C, NHALF], F32, kind="ExternalInput")
    w_d = {
        n: nc.dram_tensor(n, [C, C], F32, kind="ExternalInput")
        for n in ("wq", "wk", "wv", "wp")
    }
    b_d = {
        n: nc.dram_tensor(n, [C], F32, kind="ExternalInput")
        for n in ("bq", "bk", "bv", "bp")
    }
    gamma_d = nc.dram_tensor("gamma", [1], F32, kind="ExternalInput")
    out_d = nc.dram_tensor("out", [C, NHALF], F32, kind="ExternalOutput")
    # DRAM bounce buffers for the softmax-recip partition broadcast
    rscr_d = [
        nc.dram_tensor(f"rscr{i}", [QB], F32, kind="Internal") for i in range(4)
    ]

    x_t = x_d[:, :].rearrange("(t p) m -> t p m", p=128)
    xq_t = xq_d[:, :].rearrange("(t p) n -> t p n", p=128)
    out_t = out_d[:, :].rearrange("(t p) n -> t p n", p=128)

    with tile.TileContext(nc) as tc:
        with tc.tile_pool(name="persist", bufs=1) as pp:
            # ---------- persistent tiles ----------
            x16 = [pp.tile([128, HW], BF16, tag=f"x16_{t}", name=f"x16_{t}") for t in range(2)]
            xq16 = [pp.tile([128, NHALF], BF16, tag=f"xq16_{t}", name=f"xq16_{t}") for t in range(2)]
            xb = [pp.tile([128, NHALF], F32, tag=f"xb_{t}", name=f"xb_{t}") for t in range(2)]
            kp = [pp.tile([128, HW], BF16, tag=f"kp_{t}", name=f"kp_{t}") for t in range(2)]
            qp = [pp.tile([128, NHALF], BF16, tag=f"qp_{t}", name=f"qp_{t}") for t in range(2)]
            oh = [pp.tile([64, NHALF], BF16, tag=f"oh_{h}", name=f"oh_{h}") for h in range(HEADS)]
            vta = pp.tile([128, NCHUNK, HEADS, HD + 1], BF16, tag="vta", name="vta")
            wqT = [pp.tile([128, C], BF16, tag=f"wqT_{t}", name=f"wqT_{t}") for t in range(2)]
            wkT = [pp.tile([128, C], BF16, tag=f"wkT_{t}", name=f"wkT_{t}") for t in range(2)]
            wvT = [pp.tile([128, C], BF16, tag=f"wvT_{t}", name=f"wvT_{t}") for t in range(2)]
            wpTh = [pp.tile([64, C], BF16, tag=f"wpTh_{h}", name=f"wpTh_{h}") for h in range(HEADS)]
            bqp = [pp.tile([128, 1], F32, tag=f"bqp_{t}", name=f"bqp_{t}") for t in range(2)]
            bkp = [pp.tile([128, 1], F32, tag=f"bkp_{t}", name=f"bkp_{t}") for t in range(2)]
            bvb = pp.tile([128, C], F32, tag="bvb", name="bvb")
            gam = pp.tile([128, 1], F32, tag="gam", name="gam")
            gb = [pp.tile([128, 1], F32, tag=f"gb_{t}", name=f"gb_{t}") for t in range(2)]
            ident = pp.tile([128, 128], F32, tag="ident", name="ident")
            wdum = pp.tile([128, 512], BF16, tag="wdum", name="wdum")

            nc.vector.memset(vta[:, :, :, HD : HD + 1], 1.0)
            nc.vector.memset(wdum, 0.0)
            make_identity(nc, ident)

            # gamma broadcast to all partitions
            g_ap = gamma_d[:]
            nc.sync.dma_start(
                out=gam,
                in_=bass.AP(tensor=g_ap.tensor, offset=g_ap.offset, ap=[[0, 128], [1, 1]]),
            )
            # bv broadcast [128, C]
            bv_ap = b_d["bv"][:]
            nc.sync.dma_start(
                out=bvb,
                in_=bass.AP(
                    tensor=bv_ap.tensor, offset=bv_ap.offset, ap=[[0, 128], [1, C]]
                ),
            )
            # per-pair q/k biases (two heads per 128-partition tile)
            for t in range(2):
                bq_r = b_d["bq"][:].rearrange("(t p one) -> t p one", p=128, one=1)
                bk_r = b_d["bk"][:].rearrange("(t p one) -> t p one", p=128, one=1)
                nc.sync.dma_start(out=bqp[t], in_=bq_r[t])
                nc.sync.dma_start(out=bkp[t], in_=bk_r[t])
            bp_r = b_d["bp"][:].rearrange("(t p one) -> t p one", p=128, one=1)

            # ---------- setup: load x, cast, weights transpose ----------
            with (
                tc.tile_pool(name="setup_sb", bufs=2) as sb,
                tc.tile_pool(name="setup_ps", bufs=2, space="PSUM") as sps,
            ):
                # keep the PE busy through the DMA-bound setup so the HAM
                # clock gate reaches (and keeps) full rate before the
                # projection matmuls start
                wps = sps.tile([128, 512], F32, tag="wps", name="wps")
                for _ in range(24):
                    nc.tensor.matmul(
                        wps, lhsT=wdum[:, 0:128], rhs=wdum, start=True, stop=True
                    )

                # weights: load natural [o, c], PE-transpose to [c, o] bf16
                wT_dst = {"wq": wqT, "wk": wkT, "wv": wvT}
                for name in ("wq", "wk", "wv", "wp"):
                    wn = [sb.tile([128, C], F32, tag=f"wnat{t}", name=f"wnat{t}") for t in range(2)]
                    w_r = w_d[name][:, :].rearrange("(t p) c -> t p c", p=128)
                    for t in range(2):
                        nc.sync.dma_start(out=wn[t], in_=w_r[t])
                    for i in range(2):  # o tile
                        for j in range(2):  # c tile
                            tp = sps.tile([128, 128], F32, tag="wtp", name="wtp")
                            nc.tensor.transpose(
                                tp, wn[i][:, j * 128 : (j + 1) * 128], ident
                            )
                            if name == "wp":
                                # split to per-head base-0 tiles via DMA
                                wp_st = sb.tile([128, 128], BF16, tag="wpst", name="wpst")
                                nc.vector.tensor_copy(out=wp_st, in_=tp)
                                for hh in range(2):
                                    h = 2 * j + hh
                                    nc.sync.dma_start(
                                        out=wpTh[h][:, i * 128 : (i + 1) * 128],
                                        in_=wp_st[64 * hh : 64 * hh + 64, :],
                                    )
                            else:
                                nc.vector.tensor_copy(
                                    out=wT_dst[name][j][:, i * 128 : (i + 1) * 128],
                                    in_=tp,
                                )

                # x loads split into column chunks so they spread across DMA
                # queues and the casts overlap the remaining transfers
                for t in range(2):
                    xf = sb.tile([128, HW], F32, tag=f"xf{t}", name=f"xf{t}")
                    for b4 in range(4):
                        sl = slice(b4 * 1024, (b4 + 1) * 1024)
                        nc.sync.dma_start(out=xf[:, sl], in_=x_t[t][:, sl])
                        if t == 0:
                            nc.scalar.copy(out=x16[t][:, sl], in_=xf[:, sl])
                        else:
                            nc.vector.tensor_copy(out=x16[t][:, sl], in_=xf[:, sl])
                for t in range(2):
                    for b4 in range(2):
                        sl = slice(b4 * 1024, (b4 + 1) * 1024)
                        nc.sync.dma_start(out=xb[t][:, sl], in_=xq_t[t][:, sl])
                        nc.vector.tensor_copy(out=xq16[t][:, sl], in_=xb[t][:, sl])
                    bp_t = sb.tile([128, 1], F32, tag="bpt", name="bpt")
                    nc.sync.dma_start(out=bp_t, in_=bp_r[t])
                    nc.vector.tensor_mul(out=gb[t], in0=bp_t, in1=gam)
                    # xb = xq + gamma*bp
                    nc.vector.tensor_scalar_add(out=xb[t], in0=xb[t], scalar1=gb[t])

            # ---------- K, Q, V projections (128x128 mode) ----------
            with tc.tile_pool(name="proj_ps", bufs=3, space="PSUM") as bps:
                for t in range(2):
                    for mb in range(HW // 512):
                        ps = bps.tile([128, 512], F32, tag="pk", name="pk")
                        for ci in range(2):
                            nc.tensor.matmul(
                                ps,
                                lhsT=wkT[ci][:, 128 * t : 128 * t + 128],
                                rhs=x16[ci][:, mb * 512 : (mb + 1) * 512],
                                start=(ci == 0),
                                stop=(ci == 1),
                            )
                        nc.scalar.activation(
                            out=kp[t][:, mb * 512 : (mb + 1) * 512],
                            in_=ps,
                            func=mybir.ActivationFunctionType.Identity,
                            bias=bkp[t],
                        )
                for t in range(2):
                    for nb in range(NHALF // 512):
                        ps = bps.tile([128, 512], F32, tag="pk", name="pk")
                        for ci in range(2):
                            nc.tensor.matmul(
                                ps,
                                lhsT=wqT[ci][:, 128 * t : 128 * t + 128],
                                rhs=xq16[ci][:, nb * 512 : (nb + 1) * 512],
                                start=(ci == 0),
                                stop=(ci == 1),
                            )
                        nc.scalar.activation(
                            out=qp[t][:, nb * 512 : (nb + 1) * 512],
                            in_=ps,
                            func=mybir.ActivationFunctionType.Identity,
                            bias=bqp[t],
                        )
                for mc in range(NCHUNK):
                    ps = bps.tile([128, C], F32, tag="pv", name="pv")
                    for ci in range(2):
                        nc.tensor.matmul(
                            ps,
                            lhsT=x16[ci][:, mc * 128 : (mc + 1) * 128],
                            rhs=wvT[ci][:, :],
                            start=(ci == 0),
                            stop=(ci == 1),
                        )
                    nc.vector.tensor_add(
                        out=vta[:, mc, :, 0:HD],
                        in0=ps.rearrange("p (h d) -> p h d", h=HEADS),
                        in1=bvb.rearrange("p (h d) -> p h d", h=HEADS),
                    )

            # ---------- attention, entirely in (64,128) tile mode ----------
            with (
                tc.tile_pool(name="st_ps", bufs=2, space="PSUM") as stp,
                tc.tile_pool(name="o_ps", bufs=1, space="PSUM") as op,
                tc.tile_pool(name="attn_sb", bufs=PV_LAG + 2) as asb,
                tc.tile_pool(name="norm_sb", bufs=2) as nsb,
            ):
                def norm_thunks(c0, c1, c2, c3, ha, hb, n0, gidx):
                    """oh[h][:, n0:+QB] = (ca+cb)[0:64] / (ca+cb)[64] from the
                    SBUF-staged PV halves. Returns single-instruction thunks
                    emitted one-per-chunk mid-next-group so the work trickles
                    into the ScalarE/DVE/DMA queues without bursts."""
                    thunks = []
                    for hi, (ca, cb, h) in enumerate(((c0, c1, ha), (c2, c3, hb))):
                        scr = rscr_d[(gidx % 2) * 2 + hi]
                        tmp = nsb.tile([HD + 1, QB], F32, tag="tmp", name="tmp")
                        lnd = nsb.tile([1, QB], F32, tag="lnd", name="lnd")
                        rp = nsb.tile([1, QB], F32, tag="rp", name="rp")
                        bcs = nsb.tile([HD, QB], F32, tag="bcs", name="bcs")

                        def t_add(tmp=tmp, ca=ca, cb=cb):
                            # SBUF-only combine on the otherwise-idle GpSimd
                            nc.gpsimd.tensor_tensor(
                                out=tmp, in0=ca, in1=cb, op=mybir.AluOpType.add
                            )

                        def t_ln(lnd=lnd, tmp=tmp):
                            # 1/den = exp(-ln(den)) — both funcs live in the
                            # natural_log_exp_and_others ACT set (no reload)
                            nc.scalar.activation(
                                out=lnd,
                                in_=tmp[HD : HD + 1, :],
                                func=mybir.ActivationFunctionType.Ln,
                            )

                        def t_rp(rp=rp, lnd=lnd):
                            nc.scalar.activation(
                                out=rp,
                                in_=lnd,
                                func=mybir.ActivationFunctionType.Exp,
                                scale=-1.0,
                            )

                        def t_bc1(rp=rp, scr=scr):
                            # partition broadcast via DRAM bounce: store...
                            nc.sync.dma_start(
                                out=scr[:].rearrange("(one n) -> one n", one=1),
                                in_=rp,
                            )

                        def t_bc(bcs=bcs, scr=scr):
                            # ...then reload with a stride-0 partition AP
                            s_ap = scr[:]
                            nc.sync.dma_start(
                                out=bcs,
                                in_=bass.AP(
                                    tensor=s_ap.tensor,
                                    offset=s_ap.offset,
                                    ap=[[0, HD], [1, QB]],
                                ),
                            )

                        def t_mul(h=h, tmp=tmp, bcs=bcs):
                            nc.gpsimd.tensor_tensor(
                                out=oh[h][:, n0 : n0 + QB],
                                in0=tmp[0:HD, :],
                                in1=bcs,
                                op=mybir.AluOpType.mult,
                            )

                        thunks += [t_add, t_ln, t_rp, t_bc1, t_bc, t_mul]
                    return thunks

                pending = []
                for t in range(2):
                    ha, hb = 2 * t, 2 * t + 1
                    for nb in range(NQB):
                        n0 = nb * QB
                        oA0 = op.tile([HD + 1, QB], F32, tag="oA0", name="oA0")
                        oA1 = op.tile([HD + 1, QB], F32, tag="oA1", name="oA1")
                        oB0 = op.tile([HD + 1, QB], F32, tag="oB0", name="oB0")
                        oB1 = op.tile([HD + 1, QB], F32, tag="oB1", name="oB1")

                        def emit_pv(exb, mc):
                            # PV: keys split 64/64 across T0/T8
                            first, last = mc == 0, mc == NCHUNK - 1
                            nc.tensor.matmul(
                                oA0, lhsT=vta[0:64, mc, ha, :],
                                rhs=exb[0:64, 0:512], start=first, stop=last,
                            )
                            nc.tensor.matmul(
                                oA1, lhsT=vta[64:128, mc, ha, :],
                                rhs=exb[64:128, 0:512], start=first, stop=last,
                            )
                            nc.tensor.matmul(
                                oB0, lhsT=vta[0:64, mc, hb, :],
                                rhs=exb[0:64, 512:1024], start=first, stop=last,
                            )
                            nc.tensor.matmul(
                                oB1, lhsT=vta[64:128, mc, hb, :],
                                rhs=exb[64:128, 512:1024], start=first, stop=last,
                            )

                        # software pipeline: PV runs PV_LAG chunks behind
                        # S/exp so the in-order PE never waits on an exp —
                        # it executes earlier chunks' PVs instead
                        from collections import deque

                        inflight = deque()
                        for mc in range(NCHUNK):
                            st = stp.tile([128, 1024], F32, tag="st", name="st")
                            # S for both heads concurrently (T0 rows 0-63,
                            # T8 rows 64-127)
                            nc.tensor.matmul(
                                st[:, 0:512],
                                lhsT=kp[t][0:64, mc * 128 : (mc + 1) * 128],
                                rhs=qp[t][0:64, n0 : n0 + QB],
                                start=True,
                                stop=True,
                            )
                            nc.tensor.matmul(
                                st[:, 512:1024],
                                lhsT=kp[t][64:128, mc * 128 : (mc + 1) * 128],
                                rhs=qp[t][64:128, n0 : n0 + QB],
                                start=True,
                                stop=True,
                            )
                            # exp split across BOTH engines every chunk:
                            # ScalarE takes head a's half (table exp, bf16
                            # written through a bitcast view of the i16
                            # tile), DVE takes head b's half (Schraudolph).
                            # Halves the exp latency the PV stream waits on.
                            exi = asb.tile([128, 1024], I16, tag="exi", name="exi")
                            exb = exi.bitcast(BF16)
                            nc.scalar.activation(
                                out=exb[:, 0:512],
                                in_=st[:, 0:512],
                                func=mybir.ActivationFunctionType.Exp,
                                scale=float(SCALE),
                            )
                            nc.vector.tensor_scalar(
                                out=exi[:, 512:1024],
                                in0=st[:, 512:1024],
                                scalar1=SCH_A,
                                scalar2=SCH_B,
                                op0=mybir.AluOpType.mult,
                                op1=mybir.AluOpType.add,
                            )
                            inflight.append((exb, mc))
                            if len(inflight) > PV_LAG:
                                emit_pv(*inflight.popleft())
                            # trickle the previous group's normalize in,
                            # one instruction per chunk
                            if pending and mc >= 4:
                                pending.pop(0)()
                        while inflight:
                            emit_pv(*inflight.popleft())
                        while pending:
                            pending.pop(0)()
                        # drain the 4 ops banks to SBUF right away (ScalarE
                        # and DVE each take two, in parallel) so the next
                        # group's PV accumulation can reclaim them quickly
                        c0 = nsb.tile([HD + 1, QB], F32, tag="c0", name="c0")
                        c1 = nsb.tile([HD + 1, QB], F32, tag="c1", name="c1")
                        c2 = nsb.tile([HD + 1, QB], F32, tag="c2", name="c2")
                        c3 = nsb.tile([HD + 1, QB], F32, tag="c3", name="c3")
                        nc.scalar.copy(out=c0, in_=oA0)
                        nc.vector.tensor_copy(out=c1, in_=oA1)
                        nc.scalar.copy(out=c2, in_=oB0)
                        nc.vector.tensor_copy(out=c3, in_=oB1)
                        pending = norm_thunks(
                            c0, c1, c2, c3, ha, hb, n0, t * NQB + nb
                        )
                for th in pending:
                    th()

            # ---------- output projection + residual ((64,128) mode) ----------
            with (
                tc.tile_pool(name="proj2_ps", bufs=3, space="PSUM") as pps,
                tc.tile_pool(name="res_sb", bufs=3) as rsb,
            ):
                for oc in range(2):
                    for nb in range(NHALF // 512):
                        ps = pps.tile([128, 512], F32, tag="pp", name="pp")
                        for h in range(HEADS):
                            nc.tensor.matmul(
                                ps,
                                lhsT=wpTh[h][:, oc * 128 : (oc + 1) * 128],
                                rhs=oh[h][:, nb * 512 : (nb + 1) * 512],
                                start=(h == 0),
                                stop=(h == HEADS - 1),
                            )
                        res = rsb.tile([128, 512], F32, tag="res", name="res")
                        nc.vector.scalar_tensor_tensor(
                            out=res,
                            in0=ps,
                            scalar=gam[:, 0:1],
                            in1=xb[oc][:, nb * 512 : (nb + 1) * 512],
                            op0=mybir.AluOpType.mult,
                            op1=mybir.AluOpType.add,
                        )
                        nc.sync.dma_start(
                            out=out_t[oc, :, nb * 512 : (nb + 1) * 512], in_=res
                        )

    if fix:
        _fix_tail_drain(nc)
    return nc


_NC_CACHE = None


def _get_nc():
    global _NC_CACHE
    if _NC_CACHE is None:
        _NC_CACHE = build()
    return _NC_CACHE


def kernel(x, wq, bq, wk, bk, wv, bv, wp, bp, gamma):
    from concourse.bass_utils import run_bass_kernel_spmd

    nc = _get_nc()
    x = np.ascontiguousarray(np.asarray(x, np.float32)).reshape(B, C, HW)
    common = {
        "wq": np.ascontiguousarray(np.asarray(wq, np.float32)),
        "wk": np.ascontiguousarray(np.asarray(wk, np.float32)),
        "wv": np.ascontiguousarray(np.asarray(wv, np.float32)),
        "wp": np.ascontiguousarray(np.asarray(wp, np.float32)),
        "bq": np.ascontiguousarray(np.asarray(bq, np.float32)),
        "bk": np.ascontiguousarray(np.asarray(bk, np.float32)),
        "bv": np.ascontiguousarray(np.asarray(bv, np.float32)),
        "bp": np.ascontiguousarray(np.asarray(bp, np.float32)),
        "gamma": np.ascontiguousarray(np.asarray(gamma, np.float32)),
    }
    in_maps = []
    for core in range(8):
        b, j = core // 2, core % 2
        m = dict(common)
        m["x"] = np.ascontiguousarray(x[b])
        m["xq"] = np.ascontiguousarray(x[b][:, j * NHALF : (j + 1) * NHALF])
        in_maps.append(m)

    res = run_bass_kernel_spmd(nc, in_maps, core_ids=list(range(8)), trace=False)
    out = np.empty((B, C, HW), np.float32)
    for core in range(8):
        b, j = core // 2, core % 2
        out[b][:, j * NHALF : (j + 1) * NHALF] = res.results[core]["out"]
    return out.reshape(B, C, H, W)


# revision 29
# speedup vs baseline: 1.0533x; 1.0533x over previous
"""Trainium2 Bass kernel for MultiHeadSelfAttention2D.

Problem: x(4,256,64,64); q,k,v,proj 1x1-conv projections; 4 heads x 64 dim;
full 4096x4096 attention per (batch,head); out = gamma*proj + x.

Sharding: 8 cores = batch(4) x query-half(2). Each core computes its full
output slice out[b][:, nhalf] on-device:
  - K,V projected from full x[b]; Q from its query half only.
  - Flash-style attention, entirely in the PE's (64,128) row-tiled mode so
    the array never mode-switches mid-loop and both 64-row groups stay busy:
      * S chunk: heads 2t and 2t+1 computed CONCURRENTLY (T0 rows 0-63,
        T8 rows 64-127) into the two banks of one [128,1024] PSUM tile.
      * exp: alternates between ScalarE (table exp) and DVE (Schraudolph
        int16 bit-trick, bitcast to bf16) so neither engine bottlenecks.
      * PV: keys split 64/64 across T0/T8, accumulated in separate PSUM
        banks (ops0/ops1), combined during normalize.
      * softmax denominator: ones-row appended to V^T (M=65); reciprocal
        via exp(-ln(x)) on ScalarE (same ACT table set as exp; no switch);
        broadcast across partitions with a K-padded ones matmul (in-mode).
  - Output projection is K=64 per head (also (64,128) mode), + residual.
Host only concatenates the 8 slices.
"""

import numpy as np

import concourse.bass as bass
import concourse.mybir as mybir
import concourse.tile as tile

B, C, H, W, HEADS = 4, 256, 64, 64, 4
HD = C // HEADS  # 64
HW = H * W  # 4096
NHALF = HW // 2  # 2048
NCHUNK = HW // 128  # 32 key chunks
QB = 512  # query block
NQB = NHALF // QB  # 4
SCALE = 1.0 / np.sqrt(HD)
F32 = mybir.dt.float32
BF16 = mybir.dt.bfloat16
I16 = mybir.dt.int16

LN2 = float(np.log(2.0))
SCH_A = float(SCALE) * 128.0 / LN2  # folds the 1/sqrt(hd) score scale
SCH_B = 127.0 * 128.0 - 7.42

# exp engine split: True -> ScalarE, False -> DVE Schraudolph. Pure
# alternation keeps the group tail (chunks 28-31) strictly interleaved so no
# engine serializes two exps right where the PV tail and PSUM drain wait;
# the one extra DVE chunk for load balance sits mid-group at 14.
EXP_ON_ACT = [mc % 2 == 0 and mc != 14 for mc in range(NCHUNK)]
# chunks the PV matmuls trail behind S/exp: must cover the exp latency
# (~1.2us) with PE chunk periods (~0.7us) so the in-order PE never waits
PV_LAG = 3


def _fix_tail_drain(nc, keep=1):
    """This walrus build rejects instructions with more than a couple of
    semaphore waits. Inserting a same-engine NoOp immediately before an
    instruction is semantically identical (the engine blocks at the NoOp
    instead), so split any excess waits onto adjacent NoOps."""
    fn = nc.m.functions[0]
    for bi, blk in enumerate(fn.blocks):
        insts = list(blk.instructions)
        changed = False
        new_list = []
        for ins in insts:
            si = ins.sync_info
            if si is not None and len(si.on_wait) > keep:
                waits = list(si.on_wait)
                kept, excess = waits[:keep], waits[keep:]
                for j, w in enumerate(excess):
                    new_list.append(
                        mybir.InstNoOp(
                            name=f"waitfix-{bi}-{ins.name}-{j}",
                            engine=ins.engine,
                            sync_info=mybir.SyncInfo(on_wait=[w], on_update=[]),
                        )
                    )
                ins.sync_info = mybir.SyncInfo(on_wait=kept, on_update=si.on_update)
                changed = True
            new_list.append(ins)
        if changed:
            blk.instructions = new_list


def build(fix=True):
    from concourse.masks import make_identity

    nc = bass.Bass("TRN2", target_bir_lowering=False)

    x_d = nc.dram_tensor("x", [C, HW], F32, kind="ExternalInput")
    xq_d = nc.dram_tensor("xq", [C, NHALF], F32, kind="ExternalInput")
    w_d = {
        n: nc.dram_tensor(n, [C, C], F32, kind="ExternalInput")
        for n in ("wq", "wk", "wv", "wp")
    }
    b_d = {
        n: nc.dram_tensor(n, [C], F32, kind="ExternalInput")
        for n in ("bq", "bk", "bv", "bp")
    }
    gamma_d = nc.dram_tensor("gamma", [1], F32, kind="ExternalInput")
    out_d = nc.dram_tensor("out", [C, NHALF], F32, kind="ExternalOutput")
    # DRAM bounce buffers for the softmax-recip partition broadcast
    rscr_d = [
        nc.dram_tensor(f"rscr{i}", [QB], F32, kind="Internal") for i in range(4)
    ]

    x_t = x_d[:, :].rearrange("(t p) m -> t p m", p=128)
    xq_t = xq_d[:, :].rearrange("(t p) n -> t p n", p=128)
    out_t = out_d[:, :].rearrange("(t p) n -> t p n", p=128)

    with tile.TileContext(nc) as tc:
        with tc.tile_pool(name="persist", bufs=1) as pp:
            # ---------- persistent tiles ----------
            x16 = [pp.tile([128, HW], BF16, tag=f"x16_{t}", name=f"x16_{t}") for t in range(2)]
            xq16 = [pp.tile([128, NHALF], BF16, tag=f"xq16_{t}", name=f"xq16_{t}") for t in range(2)]
            xb = [pp.tile([128, NHALF], F32, tag=f"xb_{t}", name=f"xb_{t}") for t in range(2)]
            kp = [pp.tile([128, HW], BF16, tag=f"kp_{t}", name=f"kp_{t}") for t in range(2)]
            qp = [pp.tile([128, NHALF], BF16, tag=f"qp_{t}", name=f"qp_{t}") for t in range(2)]
            oh = [pp.tile([64, NHALF], BF16, tag=f"oh_{h}", name=f"oh_{h}") for h in range(HEADS)]
            vta = pp.tile([128, NCHUNK, HEADS, HD + 1], BF16, tag="vta", name="vta")
            wqT = [pp.tile([128, C], BF16, tag=f"wqT_{t}", name=f"wqT_{t}") for t in range(2)]
            wkT = [pp.tile([128, C], BF16, tag=f"wkT_{t}", name=f"wkT_{t}") for t in range(2)]
            wvT = [pp.tile([128, C], BF16, tag=f"wvT_{t}", name=f"wvT_{t}") for t in range(2)]
            wpTh = [pp.tile([64, C], BF16, tag=f"wpTh_{h}", name=f"wpTh_{h}") for h in range(HEADS)]
            bqp = [pp.tile([128, 1], F32, tag=f"bqp_{t}", name=f"bqp_{t}") for t in range(2)]
            bkp = [pp.tile([128, 1], F32, tag=f"bkp_{t}", name=f"bkp_{t}") for t in range(2)]
            bvb = pp.tile([128, C], F32, tag="bvb", name="bvb")
            gam = pp.tile([128, 1], F32, tag="gam", name="gam")
            gb = [pp.tile([128, 1], F32, tag=f"gb_{t}", name=f"gb_{t}") for t in range(2)]
            ident = pp.tile([128, 128], F32, tag="ident", name="ident")
            wdum = pp.tile([128, 512], BF16, tag="wdum", name="wdum")

            nc.vector.memset(vta[:, :, :, HD : HD + 1], 1.0)
            nc.vector.memset(wdum, 0.0)
            make_identity(nc, ident)

            # gamma broadcast to all partitions
            g_ap = gamma_d[:]
            nc.sync.dma_start(
                out=gam,
                in_=bass.AP(tensor=g_ap.tensor, offset=g_ap.offset, ap=[[0, 128], [1, 1]]),
            )
            # bv broadcast [128, C]
            bv_ap = b_d["bv"][:]
            nc.sync.dma_start(
                out=bvb,
                in_=bass.AP(
                    tensor=bv_ap.tensor, offset=bv_ap.offset, ap=[[0, 128], [1, C]]
                ),
            )
            # per-pair q/k biases (two heads per 128-partition tile)
            for t in range(2):
                bq_r = b_d["bq"][:].rearrange("(t p one) -> t p one", p=128, one=1)
                bk_r = b_d["bk"][:].rearrange("(t p one) -> t p one", p=128, one=1)
                nc.sync.dma_start(out=bqp[t], in_=bq_r[t])
                nc.sync.dma_start(out=bkp[t], in_=bk_r[t])
            bp_r = b_d["bp"][:].rearrange("(t p one) -> t p one", p=128, one=1)

            # ---------- setup: load x, cast, weights transpose ----------
            with (
                tc.tile_pool(name="setup_sb", bufs=2) as sb,
                tc.tile_pool(name="setup_ps", bufs=2, space="PSUM") as sps,
            ):
                # keep the PE busy through the DMA-bound setup so the HAM
                # clock gate reaches (and keeps) full rate before the
                # projection matmuls start
                wps = sps.tile([128, 512], F32, tag="wps", name="wps")
                for _ in range(24):
                    nc.tensor.matmul(
                        wps, lhsT=wdum[:, 0:128], rhs=wdum, start=True, stop=True
                    )

                # weights: load natural [o, c], PE-transpose to [c, o] bf16
                wT_dst = {"wq": wqT, "wk": wkT, "wv": wvT}
                for name in ("wq", "wk", "wv", "wp"):
                    wn = [sb.tile([128, C], F32, tag=f"wnat{t}", name=f"wnat{t}") for t in range(2)]
                    w_r = w_d[name][:, :].rearrange("(t p) c -> t p c", p=128)
                    for t in range(2):
                        nc.sync.dma_start(out=wn[t], in_=w_r[t])
                    for i in range(2):  # o tile
                        for j in range(2):  # c tile
                            tp = sps.tile([128, 128], F32, tag="wtp", name="wtp")
                            nc.tensor.transpose(
                                tp, wn[i][:, j * 128 : (j + 1) * 128], ident
                            )
                            if name == "wp":
                                # split to per-head base-0 tiles via DMA
                                wp_st = sb.tile([128, 128], BF16, tag="wpst", name="wpst")
                                nc.vector.tensor_copy(out=wp_st, in_=tp)
                                for hh in range(2):
                                    h = 2 * j + hh
                                    nc.sync.dma_start(
                                        out=wpTh[h][:, i * 128 : (i + 1) * 128],
                                        in_=wp_st[64 * hh : 64 * hh + 64, :],
                                    )
                            else:
                                nc.vector.tensor_copy(
                                    out=wT_dst[name][j][:, i * 128 : (i + 1) * 128],
                                    in_=tp,
                                )

                # x loads split into column chunks so they spread across DMA
                # queues and the casts overlap the remaining transfers
                for t in range(2):
                    xf = sb.tile([128, HW], F32, tag=f"xf{t}", name=f"xf{t}")
                    for b4 in range(4):
                        sl = slice(b4 * 1024, (b4 + 1) * 1024)
                        nc.sync.dma_start(out=xf[:, sl], in_=x_t[t][:, sl])
                        if t == 0:
                            nc.scalar.copy(out=x16[t][:, sl], in_=xf[:, sl])
                        else:
                            nc.vector.tensor_copy(out=x16[t][:, sl], in_=xf[:, sl])
                for t in range(2):
                    for b4 in range(2):
                        sl = slice(b4 * 1024, (b4 + 1) * 1024)
                        nc.sync.dma_start(out=xb[t][:, sl], in_=xq_t[t][:, sl])
                        nc.vector.tensor_copy(out=xq16[t][:, sl], in_=xb[t][:, sl])
                    bp_t = sb.tile([128, 1], F32, tag="bpt", name="bpt")
                    nc.sync.dma_start(out=bp_t, in_=bp_r[t])
                    nc.vector.tensor_mul(out=gb[t], in0=bp_t, in1=gam)
                    # xb = xq + gamma*bp
                    nc.vector.tensor_scalar_add(out=xb[t], in0=xb[t], scalar1=gb[t])

            # ---------- K, Q, V projections (128x128 mode) ----------
            with tc.tile_pool(name="proj_ps", bufs=3, space="PSUM") as bps:
                for t in range(2):
                    for mb in range(HW // 512):
                        ps = bps.tile([128, 512], F32, tag="pk", name="pk")
                        for ci in range(2):
                            nc.tensor.matmul(
                                ps,
                                lhsT=wkT[ci][:, 128 * t : 128 * t + 128],
                                rhs=x16[ci][:, mb * 512 : (mb + 1) * 512],
                                start=(ci == 0),
                                stop=(ci == 1),
                            )
                        nc.scalar.activation(
                            out=kp[t][:, mb * 512 : (mb + 1) * 512],
                            in_=ps,
                            func=mybir.ActivationFunctionType.Identity,
                            bias=bkp[t],
                        )
                for t in range(2):
                    for nb in range(NHALF // 512):
                        ps = bps.tile([128, 512], F32, tag="pk", name="pk")
                        for ci in range(2):
                            nc.tensor.matmul(
                                ps,
                                lhsT=wqT[ci][:, 128 * t : 128 * t + 128],
                                rhs=xq16[ci][:, nb * 512 : (nb + 1) * 512],
                                start=(ci == 0),
                                stop=(ci == 1),
                            )
                        nc.scalar.activation(
                            out=qp[t][:, nb * 512 : (nb + 1) * 512],
                            in_=ps,
                            func=mybir.ActivationFunctionType.Identity,
                            bias=bqp[t],
                        )
                for mc in range(NCHUNK):
                    ps = bps.tile([128, C], F32, tag="pv", name="pv")
                    for ci in range(2):
                        nc.tensor.matmul(
                            ps,
                            lhsT=x16[ci][:, mc * 128 : (mc + 1) * 128],
                            rhs=wvT[ci][:, :],
                            start=(ci == 0),
                            stop=(ci == 1),
                        )
                    nc.vector.tensor_add(
                        out=vta[:, mc, :, 0:HD],
                        in0=ps.rearrange("p (h d) -> p h d", h=HEADS),
                        in1=bvb.rearrange("p (h d) -> p h d", h=HEADS),
                    )

            # ---------- attention, entirely in (64,128) tile mode ----------
            with (
                tc.tile_pool(name="st_ps", bufs=2, space="PSUM") as stp,
                tc.tile_pool(name="o_ps", bufs=1, space="PSUM") as op,
                tc.tile_pool(name="attn_sb", bufs=PV_LAG + 2) as asb,
                tc.tile_pool(name="norm_sb", bufs=2) as nsb,
            ):
                def norm_thunks(c0, c1, c2, c3, ha, hb, n0, gidx):
                    """oh[h][:, n0:+QB] = (ca+cb)[0:64] / (ca+cb)[64] from the
                    SBUF-staged PV halves. Returns single-instruction thunks
                    emitted one-per-chunk mid-next-group so the work trickles
                    into the ScalarE/DVE/DMA queues without bursts."""
                    thunks = []
                    for hi, (ca, cb, h) in enumerate(((c0, c1, ha), (c2, c3, hb))):
                        scr = rscr_d[(gidx % 2) * 2 + hi]
                        tmp = nsb.tile([HD + 1, QB], F32, tag="tmp", name="tmp")
                        lnd = nsb.tile([1, QB], F32, tag="lnd", name="lnd")
                        rp = nsb.tile([1, QB], F32, tag="rp", name="rp")
                        bcs = nsb.tile([HD, QB], F32, tag="bcs", name="bcs")

                        def t_add(tmp=tmp, ca=ca, cb=cb):
                            # SBUF-only combine on the otherwise-idle GpSimd
                            nc.gpsimd.tensor_tensor(
                                out=tmp, in0=ca, in1=cb, op=mybir.AluOpType.add
                            )

                        def t_ln(lnd=lnd, tmp=tmp):
                            # 1/den = exp(-ln(den)) — both funcs live in the
                            # natural_log_exp_and_others ACT set (no reload)
                            nc.scalar.activation(
                                out=lnd,
                                in_=tmp[HD : HD + 1, :],
                                func=mybir.ActivationFunctionType.Ln,
                            )

                        def t_rp(rp=rp, lnd=lnd):
                            nc.scalar.activation(
                                out=rp,
                                in_=lnd,
                                func=mybir.ActivationFunctionType.Exp,
                                scale=-1.0,
                            )

                        def t_bc1(rp=rp, scr=scr):
                            # partition broadcast via DRAM bounce: store...
                            nc.sync.dma_start(
                                out=scr[:].rearrange("(one n) -> one n", one=1),
                                in_=rp,
                            )

                        def t_bc(bcs=bcs, scr=scr):
                            # ...then reload with a stride-0 partition AP
                            s_ap = scr[:]
                            nc.sync.dma_start(
                                out=bcs,
                                in_=bass.AP(
                                    tensor=s_ap.tensor,
                                    offset=s_ap.offset,
                                    ap=[[0, HD], [1, QB]],
                                ),
                            )

                        def t_mul(h=h, tmp=tmp, bcs=bcs):
                            nc.gpsimd.tensor_tensor(
                                out=oh[h][:, n0 : n0 + QB],
                                in0=tmp[0:HD, :],
                                in1=bcs,
                                op=mybir.AluOpType.mult,
                            )

                        thunks += [t_add, t_ln, t_rp, t_bc1, t_bc, t_mul]
                    return thunks

                pending = []
                for t in range(2):
                    ha, hb = 2 * t, 2 * t + 1
                    for nb in range(NQB):
                        n0 = nb * QB
                        oA0 = op.tile([HD + 1, QB], F32, tag="oA0", name="oA0")
                        oA1 = op.tile([HD + 1, QB], F32, tag="oA1", name="oA1")
                        oB0 = op.tile([HD + 1, QB], F32, tag="oB0", name="oB0")
                        oB1 = op.tile([HD + 1, QB], F32, tag="oB1", name="oB1")

                        def emit_pv(exb, mc):
                            # PV: keys split 64/64 across T0/T8
                            first, last = mc == 0, mc == NCHUNK - 1
                            nc.tensor.matmul(
                                oA0, lhsT=vta[0:64, mc, ha, :],
                                rhs=exb[0:64, 0:512], start=first, stop=last,
                            )
                            nc.tensor.matmul(
                                oA1, lhsT=vta[64:128, mc, ha, :],
                                rhs=exb[64:128, 0:512], start=first, stop=last,
                            )
                            nc.tensor.matmul(
                                oB0, lhsT=vta[0:64, mc, hb, :],
                                rhs=exb[0:64, 512:1024], start=first, stop=last,
                            )
                            nc.tensor.matmul(
                                oB1, lhsT=vta[64:128, mc, hb, :],
                                rhs=exb[64:128, 512:1024], start=first, stop=last,
                            )

                        # software pipeline: PV runs PV_LAG chunks behind
                        # S/exp so the in-order PE never waits on an exp —
                        # it executes earlier chunks' PVs instead
                        from collections import deque

                        inflight = deque()
                        for mc in range(NCHUNK):
                            st = stp.tile([128, 1024], F32, tag="st", name="st")
                            # S for both heads concurrently (T0 rows 0-63,
                            # T8 rows 64-127)
                            nc.tensor.matmul(
                                st[:, 0:512],
                                lhsT=kp[t][0:64, mc * 128 : (mc + 1) * 128],
                                rhs=qp[t][0:64, n0 : n0 + QB],
                                start=True,
                                stop=True,
                            )
                            nc.tensor.matmul(
                                st[:, 512:1024],
                                lhsT=kp[t][64:128, mc * 128 : (mc + 1) * 128],
                                rhs=qp[t][64:128, n0 : n0 + QB],
                                start=True,
                                stop=True,
                            )
                            # exp on alternating engines
                            if EXP_ON_ACT[mc]:
                                ex = asb.tile([128, 1024], BF16, tag="ex", name="ex")
                                nc.scalar.activation(
                                    out=ex,
                                    in_=st,
                                    func=mybir.ActivationFunctionType.Exp,
                                    scale=float(SCALE),
                                )
                                exb = ex
                            else:
                                exi = asb.tile([128, 1024], I16, tag="exi", name="exi")
                                nc.vector.tensor_scalar(
                                    out=exi,
                                    in0=st,
                                    scalar1=SCH_A,
                                    scalar2=SCH_B,
                                    op0=mybir.AluOpType.mult,
                                    op1=mybir.AluOpType.add,
                                )
                                exb = exi.bitcast(BF16)
                            inflight.append((exb, mc))
                            if len(inflight) > PV_LAG:
                                emit_pv(*inflight.popleft())
                            # trickle the previous group's PSUM drain (2 per
                            # chunk at mc 1-2, right after this group's first
                            # S/exp are queued) then its normalize chain
                            if pending:
                                npop = 2 if mc in (1, 2) and len(pending) > 12 else 1
                                for _ in range(min(npop, len(pending))):
                                    pending.pop(0)()
                        while inflight:
                            emit_pv(*inflight.popleft())
                        while pending:
                            pending.pop(0)()
                        # defer the 4 ops-bank drains into the next group's
                        # first chunks (emitted before its PV(0) so the bank
                        # reuse dependency stays correct)
                        c0 = nsb.tile([HD + 1, QB], F32, tag="c0", name="c0")
                        c1 = nsb.tile([HD + 1, QB], F32, tag="c1", name="c1")
                        c2 = nsb.tile([HD + 1, QB], F32, tag="c2", name="c2")
                        c3 = nsb.tile([HD + 1, QB], F32, tag="c3", name="c3")
                        drains = [
                            lambda c0=c0, o=oA0: nc.scalar.copy(out=c0, in_=o),
                            lambda c1=c1, o=oA1: nc.vector.tensor_copy(out=c1, in_=o),
                            lambda c2=c2, o=oB0: nc.scalar.copy(out=c2, in_=o),
                            lambda c3=c3, o=oB1: nc.vector.tensor_copy(out=c3, in_=o),
                        ]
                        pending = drains + norm_thunks(
                            c0, c1, c2, c3, ha, hb, n0, t * NQB + nb
                        )
                for th in pending:
                    th()

            # ---------- output projection + residual ((64,128) mode) ----------
            with (
                tc.tile_pool(name="proj2_ps", bufs=3, space="PSUM") as pps,
                tc.tile_pool(name="res_sb", bufs=3) as rsb,
            ):
                for oc in range(2):
                    for nb in range(NHALF // 512):
                        ps = pps.tile([128, 512], F32, tag="pp", name="pp")
                        for h in range(HEADS):
                            nc.tensor.matmul(
                                ps,
                                lhsT=wpTh[h][:, oc * 128 : (oc + 1) * 128],
                                rhs=oh[h][:, nb * 512 : (nb + 1) * 512],
                                start=(h == 0),
                                stop=(h == HEADS - 1),
                            )
                        res = rsb.tile([128, 512], F32, tag="res", name="res")
                        nc.vector.scalar_tensor_tensor(
                            out=res,
                            in0=ps,
                            scalar=gam[:, 0:1],
                            in1=xb[oc][:, nb * 512 : (nb + 1) * 512],
                            op0=mybir.AluOpType.mult,
                            op1=mybir.AluOpType.add,
                        )
                        nc.sync.dma_start(
                            out=out_t[oc, :, nb * 512 : (nb + 1) * 512], in_=res
                        )

    if fix:
        _fix_tail_drain(nc)
    return nc


_NC_CACHE = None


def _get_nc():
    global _NC_CACHE
    if _NC_CACHE is None:
        _NC_CACHE = build()
    return _NC_CACHE


def kernel(x, wq, bq, wk, bk, wv, bv, wp, bp, gamma):
    from concourse.bass_utils import run_bass_kernel_spmd

    nc = _get_nc()
    x = np.ascontiguousarray(np.asarray(x, np.float32)).reshape(B, C, HW)
    common = {
        "wq": np.ascontiguousarray(np.asarray(wq, np.float32)),
        "wk": np.ascontiguousarray(np.asarray(wk, np.float32)),
        "wv": np.ascontiguousarray(np.asarray(wv, np.float32)),
        "wp": np.ascontiguousarray(np.asarray(wp, np.float32)),
        "bq": np.ascontiguousarray(np.asarray(bq, np.float32)),
        "bk": np.ascontiguousarray(np.asarray(bk, np.float32)),
        "bv": np.ascontiguousarray(np.asarray(bv, np.float32)),
        "bp": np.ascontiguousarray(np.asarray(bp, np.float32)),
        "gamma": np.ascontiguousarray(np.asarray(gamma, np.float32)),
    }
    in_maps = []
    for core in range(8):
        b, j = core // 2, core % 2
        m = dict(common)
        m["x"] = np.ascontiguousarray(x[b])
        m["xq"] = np.ascontiguousarray(x[b][:, j * NHALF : (j + 1) * NHALF])
        in_maps.append(m)

    res = run_bass_kernel_spmd(nc, in_maps, core_ids=list(range(8)), trace=False)
    out = np.empty((B, C, HW), np.float32)
    for core in range(8):
        b, j = core // 2, core % 2
        out[b][:, j * NHALF : (j + 1) * NHALF] = res.results[core]["out"]
    return out.reshape(B, C, H, W)


# revision 35
# speedup vs baseline: 1.1101x; 1.0539x over previous
"""Trainium2 Bass kernel for MultiHeadSelfAttention2D.

Problem: x(4,256,64,64); q,k,v,proj 1x1-conv projections; 4 heads x 64 dim;
full 4096x4096 attention per (batch,head); out = gamma*proj + x.

Sharding: 8 cores = batch(4) x query-half(2). Each core computes its full
output slice out[b][:, nhalf] on-device:
  - K,V projected from full x[b]; Q from its query half only.
  - Flash-style attention, entirely in the PE's (64,128) row-tiled mode so
    the array never mode-switches mid-loop and both 64-row groups stay busy:
      * S chunk: heads 2t and 2t+1 computed CONCURRENTLY (T0 rows 0-63,
        T8 rows 64-127) into the two banks of one [128,1024] PSUM tile.
      * exp: alternates between ScalarE (table exp) and DVE (Schraudolph
        int16 bit-trick, bitcast to bf16) so neither engine bottlenecks.
      * PV: keys split 64/64 across T0/T8, accumulated in separate PSUM
        banks (ops0/ops1), combined during normalize.
      * softmax denominator: ones-row appended to V^T (M=65); reciprocal
        via exp(-ln(x)) on ScalarE (same ACT table set as exp; no switch);
        broadcast across partitions with a K-padded ones matmul (in-mode).
  - Output projection is K=64 per head (also (64,128) mode), + residual.
Host only concatenates the 8 slices.
"""

import numpy as np

import concourse.bass as bass
import concourse.mybir as mybir
import concourse.tile as tile

B, C, H, W, HEADS = 4, 256, 64, 64, 4
HD = C // HEADS  # 64
HW = H * W  # 4096
NHALF = HW // 2  # 2048
NCHUNK = HW // 128  # 32 key chunks
QB = 512  # query block
NQB = NHALF // QB  # 4
SCALE = 1.0 / np.sqrt(HD)
F32 = mybir.dt.float32
BF16 = mybir.dt.bfloat16
I16 = mybir.dt.int16

LN2 = float(np.log(2.0))
SCH_A = float(SCALE) * 128.0 / LN2  # folds the 1/sqrt(hd) score scale
SCH_B = 127.0 * 128.0 - 7.42

# exp engine split: True -> ScalarE, False -> DVE Schraudolph. Pure
# alternation keeps the group tail (chunks 28-31) strictly interleaved so no
# engine serializes two exps right where the PV tail and PSUM drain wait;
# one extra ScalarE chunk mid-group (15) balances DVE's slower per-tile exp.
EXP_ON_ACT = [mc % 2 == 0 or mc == 15 for mc in range(NCHUNK)]

# fast-reciprocal magic: 1/x ~= bitcast(C - bits(x)); C - i == (i ^ -1) + C+1
# so it runs as one int32 tensor_scalar (xor, then add). ~4% max rel error on
# the softmax denominator, which only perturbs the final output by ~0.1%.
RECIP_MAGIC_P1 = 0x7EF127EB
# chunks the PV matmuls trail behind S/exp: must cover the exp latency
# (~1.2us) with PE chunk periods (~0.7us) so the in-order PE never waits
PV_LAG = 3


def _fix_tail_drain(nc, keep=1):
    """This walrus build rejects instructions with more than a couple of
    semaphore waits. Inserting a same-engine NoOp immediately before an
    instruction is semantically identical (the engine blocks at the NoOp
    instead), so split any excess waits onto adjacent NoOps."""
    fn = nc.m.functions[0]
    for bi, blk in enumerate(fn.blocks):
        insts = list(blk.instructions)
        changed = False
        new_list = []
        for ins in insts:
            si = ins.sync_info
            if si is not None and len(si.on_wait) > keep:
                waits = list(si.on_wait)
                kept, excess = waits[:keep], waits[keep:]
                for j, w in enumerate(excess):
                    new_list.append(
                        mybir.InstNoOp(
                            name=f"waitfix-{bi}-{ins.name}-{j}",
                            engine=ins.engine,
                            sync_info=mybir.SyncInfo(on_wait=[w], on_update=[]),
                        )
                    )
                ins.sync_info = mybir.SyncInfo(on_wait=kept, on_update=si.on_update)
                changed = True
            new_list.append(ins)
        if changed:
            blk.instructions = new_list


def build(fix=True):
    from concourse.masks import make_identity

    nc = bass.Bass("TRN2", target_bir_lowering=False)

    x_d = nc.dram_tensor("x", [C, HW], F32, kind="ExternalInput")
    xq_d = nc.dram_tensor("xq", [C, NHALF], F32, kind="ExternalInput")
    w_d = {
        n: nc.dram_tensor(n, [C, C], F32, kind="ExternalInput")
        for n in ("wq", "wk", "wv", "wp")
    }
    b_d = {
        n: nc.dram_tensor(n, [C], F32, kind="ExternalInput")
        for n in ("bq", "bk", "bv", "bp")
    }
    gamma_d = nc.dram_tensor("gamma", [1], F32, kind="ExternalInput")
    out_d = nc.dram_tensor("out", [C, NHALF], F32, kind="ExternalOutput")
    # DRAM bounce buffers for the softmax-recip partition broadcast
    rscr_d = [
        nc.dram_tensor(f"rscr{i}", [QB], F32, kind="Internal") for i in range(4)
    ]

    x_t = x_d[:, :].rearrange("(t p) m -> t p m", p=128)
    xq_t = xq_d[:, :].rearrange("(t p) n -> t p n", p=128)
    out_t = out_d[:, :].rearrange("(t p) n -> t p n", p=128)

    with tile.TileContext(nc) as tc:
        with tc.tile_pool(name="persist", bufs=1) as pp:
            # ---------- persistent tiles ----------
            x16 = [pp.tile([128, HW], BF16, tag=f"x16_{t}", name=f"x16_{t}") for t in range(2)]
            xq16 = [pp.tile([128, NHALF], BF16, tag=f"xq16_{t}", name=f"xq16_{t}") for t in range(2)]
            xb = [pp.tile([128, NHALF], F32, tag=f"xb_{t}", name=f"xb_{t}") for t in range(2)]
            kp = [pp.tile([128, HW], BF16, tag=f"kp_{t}", name=f"kp_{t}") for t in range(2)]
            qp = [pp.tile([128, NHALF], BF16, tag=f"qp_{t}", name=f"qp_{t}") for t in range(2)]
            oh = [pp.tile([64, NHALF], BF16, tag=f"oh_{h}", name=f"oh_{h}") for h in range(HEADS)]
            vta = pp.tile([128, NCHUNK, HEADS, HD + 1], BF16, tag="vta", name="vta")
            wqT = [pp.tile([128, C], BF16, tag=f"wqT_{t}", name=f"wqT_{t}") for t in range(2)]
            wkT = [pp.tile([128, C], BF16, tag=f"wkT_{t}", name=f"wkT_{t}") for t in range(2)]
            wvT = [pp.tile([128, C], BF16, tag=f"wvT_{t}", name=f"wvT_{t}") for t in range(2)]
            wpTh = [pp.tile([64, C], BF16, tag=f"wpTh_{h}", name=f"wpTh_{h}") for h in range(HEADS)]
            bqp = [pp.tile([128, 1], F32, tag=f"bqp_{t}", name=f"bqp_{t}") for t in range(2)]
            bkp = [pp.tile([128, 1], F32, tag=f"bkp_{t}", name=f"bkp_{t}") for t in range(2)]
            bvb = pp.tile([128, C], F32, tag="bvb", name="bvb")
            gam = pp.tile([128, 1], F32, tag="gam", name="gam")
            gb = [pp.tile([128, 1], F32, tag=f"gb_{t}", name=f"gb_{t}") for t in range(2)]
            ident = pp.tile([128, 128], F32, tag="ident", name="ident")
            wdum = pp.tile([128, 512], BF16, tag="wdum", name="wdum")

            nc.vector.memset(vta[:, :, :, HD : HD + 1], 1.0)
            nc.vector.memset(wdum, 0.0)
            make_identity(nc, ident)

            # gamma broadcast to all partitions
            g_ap = gamma_d[:]
            nc.sync.dma_start(
                out=gam,
                in_=bass.AP(tensor=g_ap.tensor, offset=g_ap.offset, ap=[[0, 128], [1, 1]]),
            )
            # bv broadcast [128, C]
            bv_ap = b_d["bv"][:]
            nc.sync.dma_start(
                out=bvb,
                in_=bass.AP(
                    tensor=bv_ap.tensor, offset=bv_ap.offset, ap=[[0, 128], [1, C]]
                ),
            )
            # per-pair q/k biases (two heads per 128-partition tile)
            for t in range(2):
                bq_r = b_d["bq"][:].rearrange("(t p one) -> t p one", p=128, one=1)
                bk_r = b_d["bk"][:].rearrange("(t p one) -> t p one", p=128, one=1)
                nc.sync.dma_start(out=bqp[t], in_=bq_r[t])
                nc.sync.dma_start(out=bkp[t], in_=bk_r[t])
            bp_r = b_d["bp"][:].rearrange("(t p one) -> t p one", p=128, one=1)

            # ---------- setup: load x, cast, weights transpose ----------
            with (
                tc.tile_pool(name="setup_sb", bufs=2) as sb,
                tc.tile_pool(name="setup_ps", bufs=2, space="PSUM") as sps,
            ):
                # keep the PE busy through the DMA-bound setup so the HAM
                # clock gate reaches (and keeps) full rate before the
                # projection matmuls start
                wps = sps.tile([128, 512], F32, tag="wps", name="wps")
                for _ in range(24):
                    nc.tensor.matmul(
                        wps, lhsT=wdum[:, 0:128], rhs=wdum, start=True, stop=True
                    )

                # weights: load natural [o, c], PE-transpose to [c, o] bf16
                wT_dst = {"wq": wqT, "wk": wkT, "wv": wvT}
                for name in ("wq", "wk", "wv", "wp"):
                    wn = [sb.tile([128, C], F32, tag=f"wnat{t}", name=f"wnat{t}") for t in range(2)]
                    w_r = w_d[name][:, :].rearrange("(t p) c -> t p c", p=128)
                    for t in range(2):
                        nc.sync.dma_start(out=wn[t], in_=w_r[t])
                    for i in range(2):  # o tile
                        for j in range(2):  # c tile
                            tp = sps.tile([128, 128], F32, tag="wtp", name="wtp")
                            nc.tensor.transpose(
                                tp, wn[i][:, j * 128 : (j + 1) * 128], ident
                            )
                            if name == "wp":
                                # split to per-head base-0 tiles via DMA
                                wp_st = sb.tile([128, 128], BF16, tag="wpst", name="wpst")
                                nc.vector.tensor_copy(out=wp_st, in_=tp)
                                for hh in range(2):
                                    h = 2 * j + hh
                                    nc.sync.dma_start(
                                        out=wpTh[h][:, i * 128 : (i + 1) * 128],
                                        in_=wp_st[64 * hh : 64 * hh + 64, :],
                                    )
                            else:
                                nc.vector.tensor_copy(
                                    out=wT_dst[name][j][:, i * 128 : (i + 1) * 128],
                                    in_=tp,
                                )

                # x loads split into column chunks so they spread across DMA
                # queues and the casts overlap the remaining transfers
                for t in range(2):
                    xf = sb.tile([128, HW], F32, tag=f"xf{t}", name=f"xf{t}")
                    for b4 in range(4):
                        sl = slice(b4 * 1024, (b4 + 1) * 1024)
                        nc.sync.dma_start(out=xf[:, sl], in_=x_t[t][:, sl])
                        if t == 0:
                            nc.scalar.copy(out=x16[t][:, sl], in_=xf[:, sl])
                        else:
                            nc.vector.tensor_copy(out=x16[t][:, sl], in_=xf[:, sl])
                for t in range(2):
                    for b4 in range(2):
                        sl = slice(b4 * 1024, (b4 + 1) * 1024)
                        nc.sync.dma_start(out=xb[t][:, sl], in_=xq_t[t][:, sl])
                        nc.vector.tensor_copy(out=xq16[t][:, sl], in_=xb[t][:, sl])
                    bp_t = sb.tile([128, 1], F32, tag="bpt", name="bpt")
                    nc.sync.dma_start(out=bp_t, in_=bp_r[t])
                    nc.vector.tensor_mul(out=gb[t], in0=bp_t, in1=gam)
                    # xb = xq + gamma*bp
                    nc.vector.tensor_scalar_add(out=xb[t], in0=xb[t], scalar1=gb[t])

            # ---------- K, Q, V projections (128x128 mode) ----------
            with tc.tile_pool(name="proj_ps", bufs=3, space="PSUM") as bps:
                for t in range(2):
                    for mb in range(HW // 512):
                        ps = bps.tile([128, 512], F32, tag="pk", name="pk")
                        for ci in range(2):
                            nc.tensor.matmul(
                                ps,
                                lhsT=wkT[ci][:, 128 * t : 128 * t + 128],
                                rhs=x16[ci][:, mb * 512 : (mb + 1) * 512],
                                start=(ci == 0),
                                stop=(ci == 1),
                            )
                        nc.scalar.activation(
                            out=kp[t][:, mb * 512 : (mb + 1) * 512],
                            in_=ps,
                            func=mybir.ActivationFunctionType.Identity,
                            bias=bkp[t],
                        )
                for t in range(2):
                    for nb in range(NHALF // 512):
                        ps = bps.tile([128, 512], F32, tag="pk", name="pk")
                        for ci in range(2):
                            nc.tensor.matmul(
                                ps,
                                lhsT=wqT[ci][:, 128 * t : 128 * t + 128],
                                rhs=xq16[ci][:, nb * 512 : (nb + 1) * 512],
                                start=(ci == 0),
                                stop=(ci == 1),
                            )
                        nc.scalar.activation(
                            out=qp[t][:, nb * 512 : (nb + 1) * 512],
                            in_=ps,
                            func=mybir.ActivationFunctionType.Identity,
                            bias=bqp[t],
                        )
                for mc in range(NCHUNK):
                    ps = bps.tile([128, C], F32, tag="pv", name="pv")
                    for ci in range(2):
                        nc.tensor.matmul(
                            ps,
                            lhsT=x16[ci][:, mc * 128 : (mc + 1) * 128],
                            rhs=wvT[ci][:, :],
                            start=(ci == 0),
                            stop=(ci == 1),
                        )
                    nc.vector.tensor_add(
                        out=vta[:, mc, :, 0:HD],
                        in0=ps.rearrange("p (h d) -> p h d", h=HEADS),
                        in1=bvb.rearrange("p (h d) -> p h d", h=HEADS),
                    )

            # ---------- attention, entirely in (64,128) tile mode ----------
            with (
                tc.tile_pool(name="st_ps", bufs=2, space="PSUM") as stp,
                tc.tile_pool(name="o_ps", bufs=1, space="PSUM") as op,
                tc.tile_pool(name="attn_sb", bufs=PV_LAG + 2) as asb,
                tc.tile_pool(name="norm_sb", bufs=2) as nsb,
            ):
                def norm_thunks(cA, cB, ha, hb, n0, gidx):
                    """oh[h][:, n0:+QB] = (lo+hi)[0:64] / (lo+hi)[64] from the
                    SBUF-staged [65,1024] drains. Everything runs on GpSimd
                    (add, int-trick reciprocal, scale) and DMA (partition
                    broadcast) so the exp engines stay exp-only."""
                    thunks = []
                    for hi, (cc, h) in enumerate(((cA, ha), (cB, hb))):
                        scr = rscr_d[(gidx % 2) * 2 + hi]
                        tmp = nsb.tile([HD + 1, QB], F32, tag="tmp", name="tmp")
                        rp = nsb.tile([1, QB], mybir.dt.int32, tag="rp", name="rp")
                        bcs = nsb.tile([HD, QB], F32, tag="bcs", name="bcs")

                        def t_add(tmp=tmp, cc=cc):
                            nc.gpsimd.tensor_tensor(
                                out=tmp,
                                in0=cc[:, 0:QB],
                                in1=cc[:, QB : 2 * QB],
                                op=mybir.AluOpType.add,
                            )

                        def t_rp(rp=rp, tmp=tmp):
                            # C - bits(x) as (bits(x) * -1) + C (both arith
                            # ops; bitwise+arith can't mix in one instr)
                            nc.gpsimd.tensor_scalar(
                                out=rp,
                                in0=tmp[HD : HD + 1, :].bitcast(mybir.dt.int32),
                                scalar1=-1,
                                scalar2=RECIP_MAGIC_P1 - 1,
                                op0=mybir.AluOpType.mult,
                                op1=mybir.AluOpType.add,
                            )

                        def t_bc1(rp=rp, scr=scr):
                            # partition broadcast via DRAM bounce: store...
                            nc.sync.dma_start(
                                out=scr[:].rearrange("(one n) -> one n", one=1),
                                in_=rp.bitcast(F32),
                            )

                        def t_bc(bcs=bcs, scr=scr):
                            # ...then reload with a stride-0 partition AP
                            s_ap = scr[:]
                            nc.sync.dma_start(
                                out=bcs,
                                in_=bass.AP(
                                    tensor=s_ap.tensor,
                                    offset=s_ap.offset,
                                    ap=[[0, HD], [1, QB]],
                                ),
                            )

                        def t_mul(h=h, tmp=tmp, bcs=bcs):
                            nc.gpsimd.tensor_tensor(
                                out=oh[h][:, n0 : n0 + QB],
                                in0=tmp[0:HD, :],
                                in1=bcs,
                                op=mybir.AluOpType.mult,
                            )

                        thunks += [t_add, t_rp, t_bc1, t_bc, t_mul]
                    return thunks

                pending = []
                for t in range(2):
                    ha, hb = 2 * t, 2 * t + 1
                    for nb in range(NQB):
                        n0 = nb * QB
                        # per head: one [65,1024] tile spanning two PSUM banks
                        # (lo-keys half in cols 0:512, hi-keys in 512:1024) so
                        # ONE copy per head drains the whole accumulator
                        oA = op.tile([HD + 1, 2 * QB], F32, tag="oA", name="oA")
                        oB = op.tile([HD + 1, 2 * QB], F32, tag="oB", name="oB")

                        def emit_pv(exb, mc):
                            # PV: keys split 64/64 across T0/T8
                            first, last = mc == 0, mc == NCHUNK - 1
                            nc.tensor.matmul(
                                oA[:, 0:QB], lhsT=vta[0:64, mc, ha, :],
                                rhs=exb[0:64, 0:512], start=first, stop=last,
                            )
                            nc.tensor.matmul(
                                oA[:, QB : 2 * QB], lhsT=vta[64:128, mc, ha, :],
                                rhs=exb[64:128, 0:512], start=first, stop=last,
                            )
                            nc.tensor.matmul(
                                oB[:, 0:QB], lhsT=vta[0:64, mc, hb, :],
                                rhs=exb[0:64, 512:1024], start=first, stop=last,
                            )
                            nc.tensor.matmul(
                                oB[:, QB : 2 * QB], lhsT=vta[64:128, mc, hb, :],
                                rhs=exb[64:128, 512:1024], start=first, stop=last,
                            )

                        # software pipeline: PV runs PV_LAG chunks behind
                        # S/exp so the in-order PE never waits on an exp —
                        # it executes earlier chunks' PVs instead
                        from collections import deque

                        inflight = deque()
                        for mc in range(NCHUNK):
                            st = stp.tile([128, 1024], F32, tag="st", name="st")
                            # S for both heads concurrently (T0 rows 0-63,
                            # T8 rows 64-127)
                            nc.tensor.matmul(
                                st[:, 0:512],
                                lhsT=kp[t][0:64, mc * 128 : (mc + 1) * 128],
                                rhs=qp[t][0:64, n0 : n0 + QB],
                                start=True,
                                stop=True,
                            )
                            nc.tensor.matmul(
                                st[:, 512:1024],
                                lhsT=kp[t][64:128, mc * 128 : (mc + 1) * 128],
                                rhs=qp[t][64:128, n0 : n0 + QB],
                                start=True,
                                stop=True,
                            )
                            # exp on alternating engines
                            if EXP_ON_ACT[mc]:
                                ex = asb.tile([128, 1024], BF16, tag="ex", name="ex")
                                nc.scalar.activation(
                                    out=ex,
                                    in_=st,
                                    func=mybir.ActivationFunctionType.Exp,
                                    scale=float(SCALE),
                                )
                                exb = ex
                            else:
                                exi = asb.tile([128, 1024], I16, tag="exi", name="exi")
                                nc.vector.tensor_scalar(
                                    out=exi,
                                    in0=st,
                                    scalar1=SCH_A,
                                    scalar2=SCH_B,
                                    op0=mybir.AluOpType.mult,
                                    op1=mybir.AluOpType.add,
                                )
                                exb = exi.bitcast(BF16)
                            inflight.append((exb, mc))
                            if len(inflight) > PV_LAG:
                                emit_pv(*inflight.popleft())
                            # trickle the previous group's PSUM drains then
                            # its normalize chain, one per chunk
                            if pending:
                                pending.pop(0)()
                        while inflight:
                            emit_pv(*inflight.popleft())
                        while pending:
                            pending.pop(0)()
                        # defer the two ops drains into the next group's
                        # first chunks (emitted before its PV(0) so the bank
                        # reuse dependency stays correct)
                        cA = nsb.tile([HD + 1, 2 * QB], F32, tag="cA", name="cA")
                        cB = nsb.tile([HD + 1, 2 * QB], F32, tag="cB", name="cB")
                        drains = [
                            lambda cA=cA, o=oA: nc.scalar.copy(out=cA, in_=o),
                            lambda cB=cB, o=oB: nc.vector.tensor_copy(out=cB, in_=o),
                        ]
                        pending = drains + norm_thunks(
                            cA, cB, ha, hb, n0, t * NQB + nb
                        )
                for th in pending:
                    th()

            # ---------- output projection + residual ((64,128) mode) ----------
            with (
                tc.tile_pool(name="proj2_ps", bufs=3, space="PSUM") as pps,
                tc.tile_pool(name="res_sb", bufs=3) as rsb,
            ):
                for oc in range(2):
                    for nb in range(NHALF // 512):
                        ps = pps.tile([128, 512], F32, tag="pp", name="pp")
                        for h in range(HEADS):
                            nc.tensor.matmul(
                                ps,
                                lhsT=wpTh[h][:, oc * 128 : (oc + 1) * 128],
                                rhs=oh[h][:, nb * 512 : (nb + 1) * 512],
                                start=(h == 0),
                                stop=(h == HEADS - 1),
                            )
                        res = rsb.tile([128, 512], F32, tag="res", name="res")
                        nc.vector.scalar_tensor_tensor(
                            out=res,
                            in0=ps,
                            scalar=gam[:, 0:1],
                            in1=xb[oc][:, nb * 512 : (nb + 1) * 512],
                            op0=mybir.AluOpType.mult,
                            op1=mybir.AluOpType.add,
                        )
                        nc.sync.dma_start(
                            out=out_t[oc, :, nb * 512 : (nb + 1) * 512], in_=res
                        )

    if fix:
        _fix_tail_drain(nc)
    return nc


_NC_CACHE = None


def _get_nc():
    global _NC_CACHE
    if _NC_CACHE is None:
        _NC_CACHE = build()
    return _NC_CACHE


def kernel(x, wq, bq, wk, bk, wv, bv, wp, bp, gamma):
    from concourse.bass_utils import run_bass_kernel_spmd

    nc = _get_nc()
    x = np.ascontiguousarray(np.asarray(x, np.float32)).reshape(B, C, HW)
    common = {
        "wq": np.ascontiguousarray(np.asarray(wq, np.float32)),
        "wk": np.ascontiguousarray(np.asarray(wk, np.float32)),
        "wv": np.ascontiguousarray(np.asarray(wv, np.float32)),
        "wp": np.ascontiguousarray(np.asarray(wp, np.float32)),
        "bq": np.ascontiguousarray(np.asarray(bq, np.float32)),
        "bk": np.ascontiguousarray(np.asarray(bk, np.float32)),
        "bv": np.ascontiguousarray(np.asarray(bv, np.float32)),
        "bp": np.ascontiguousarray(np.asarray(bp, np.float32)),
        "gamma": np.ascontiguousarray(np.asarray(gamma, np.float32)),
    }
    in_maps = []
    for core in range(8):
        b, j = core // 2, core % 2
        m = dict(common)
        m["x"] = np.ascontiguousarray(x[b])
        m["xq"] = np.ascontiguousarray(x[b][:, j * NHALF : (j + 1) * NHALF])
        in_maps.append(m)

    res = run_bass_kernel_spmd(nc, in_maps, core_ids=list(range(8)), trace=False)
    out = np.empty((B, C, HW), np.float32)
    for core in range(8):
        b, j = core // 2, core % 2
        out[b][:, j * NHALF : (j + 1) * NHALF] = res.results[core]["out"]
    return out.reshape(B, C, H, W)


# revision 38
# speedup vs baseline: 1.1393x; 1.0263x over previous
"""Trainium2 Bass kernel for MultiHeadSelfAttention2D.

Problem: x(4,256,64,64); q,k,v,proj 1x1-conv projections; 4 heads x 64 dim;
full 4096x4096 attention per (batch,head); out = gamma*proj + x.

Sharding: 8 cores = batch(4) x query-half(2). Each core computes its full
output slice out[b][:, nhalf] on-device:
  - K,V projected from full x[b]; Q from its query half only.
  - Flash-style attention, entirely in the PE's (64,128) row-tiled mode so
    the array never mode-switches mid-loop and both 64-row groups stay busy:
      * S chunk: heads 2t and 2t+1 computed CONCURRENTLY (T0 rows 0-63,
        T8 rows 64-127) into the two banks of one [128,1024] PSUM tile.
      * exp: alternates between ScalarE (table exp) and DVE (Schraudolph
        int16 bit-trick, bitcast to bf16) so neither engine bottlenecks.
      * PV: keys split 64/64 across T0/T8, accumulated in separate PSUM
        banks (ops0/ops1), combined during normalize.
      * softmax denominator: ones-row appended to V^T (M=65); reciprocal
        via exp(-ln(x)) on ScalarE (same ACT table set as exp; no switch);
        broadcast across partitions with a K-padded ones matmul (in-mode).
  - Output projection is K=64 per head (also (64,128) mode), + residual.
Host only concatenates the 8 slices.
"""

import numpy as np

import concourse.bass as bass
import concourse.mybir as mybir
import concourse.tile as tile

B, C, H, W, HEADS = 4, 256, 64, 64, 4
HD = C // HEADS  # 64
HW = H * W  # 4096
NHALF = HW // 2  # 2048
NCHUNK = HW // 128  # 32 key chunks
QB = 512  # query block
NQB = NHALF // QB  # 4
SCALE = 1.0 / np.sqrt(HD)
F32 = mybir.dt.float32
BF16 = mybir.dt.bfloat16
I16 = mybir.dt.int16

LN2 = float(np.log(2.0))
SCH_A = float(SCALE) * 128.0 / LN2  # folds the 1/sqrt(hd) score scale
SCH_B = 127.0 * 128.0 - 7.42

# exp engine split: True -> ScalarE, False -> DVE Schraudolph. Pure
# alternation keeps the group tail (chunks 28-31) strictly interleaved so no
# engine serializes two exps right where the PV tail and PSUM drain wait;
# one extra ScalarE chunk mid-group (15) balances DVE's slower per-tile exp.
EXP_ON_ACT = [mc % 2 == 0 or mc == 15 for mc in range(NCHUNK)]

# fast-reciprocal magic: 1/x ~= bitcast(C - bits(x)); C - i == (i ^ -1) + C+1
# so it runs as one int32 tensor_scalar (xor, then add). ~4% max rel error on
# the softmax denominator, which only perturbs the final output by ~0.1%.
RECIP_MAGIC_P1 = 0x7EF127EB
# chunks the PV matmuls trail behind S/exp: must cover the exp latency
# (~1.2us) with PE chunk periods (~0.7us) so the in-order PE never waits;
# the PV tail after each group also serves as PE filler over the PSUM drain
PV_LAG = 4


def _fix_tail_drain(nc, keep=1):
    """This walrus build rejects instructions with more than a couple of
    semaphore waits. Inserting a same-engine NoOp immediately before an
    instruction is semantically identical (the engine blocks at the NoOp
    instead), so split any excess waits onto adjacent NoOps."""
    fn = nc.m.functions[0]
    for bi, blk in enumerate(fn.blocks):
        insts = list(blk.instructions)
        changed = False
        new_list = []
        for ins in insts:
            si = ins.sync_info
            if si is not None and len(si.on_wait) > keep:
                waits = list(si.on_wait)
                kept, excess = waits[:keep], waits[keep:]
                for j, w in enumerate(excess):
                    new_list.append(
                        mybir.InstNoOp(
                            name=f"waitfix-{bi}-{ins.name}-{j}",
                            engine=ins.engine,
                            sync_info=mybir.SyncInfo(on_wait=[w], on_update=[]),
                        )
                    )
                ins.sync_info = mybir.SyncInfo(on_wait=kept, on_update=si.on_update)
                changed = True
            new_list.append(ins)
        if changed:
            blk.instructions = new_list


def build(fix=True):
    from concourse.masks import make_identity

    nc = bass.Bass("TRN2", target_bir_lowering=False)

    x_d = nc.dram_tensor("x", [C, HW], F32, kind="ExternalInput")
    xq_d = nc.dram_tensor("xq", [C, NHALF], F32, kind="ExternalInput")
    w_d = {
        n: nc.dram_tensor(n, [C, C], F32, kind="ExternalInput")
        for n in ("wq", "wk", "wv", "wp")
    }
    b_d = {
        n: nc.dram_tensor(n, [C], F32, kind="ExternalInput")
        for n in ("bq", "bk", "bv", "bp")
    }
    gamma_d = nc.dram_tensor("gamma", [1], F32, kind="ExternalInput")
    out_d = nc.dram_tensor("out", [C, NHALF], F32, kind="ExternalOutput")
    # DRAM bounce buffers for the softmax-recip partition broadcast
    rscr_d = [
        nc.dram_tensor(f"rscr{i}", [QB], F32, kind="Internal") for i in range(4)
    ]

    x_t = x_d[:, :].rearrange("(t p) m -> t p m", p=128)
    xq_t = xq_d[:, :].rearrange("(t p) n -> t p n", p=128)
    out_t = out_d[:, :].rearrange("(t p) n -> t p n", p=128)

    with tile.TileContext(nc) as tc:
        with tc.tile_pool(name="persist", bufs=1) as pp:
            # ---------- persistent tiles ----------
            x16 = [pp.tile([128, HW], BF16, tag=f"x16_{t}", name=f"x16_{t}") for t in range(2)]
            xq16 = [pp.tile([128, NHALF], BF16, tag=f"xq16_{t}", name=f"xq16_{t}") for t in range(2)]
            xb = [pp.tile([128, NHALF], F32, tag=f"xb_{t}", name=f"xb_{t}") for t in range(2)]
            kp = [pp.tile([128, HW], BF16, tag=f"kp_{t}", name=f"kp_{t}") for t in range(2)]
            qp = [pp.tile([128, NHALF], BF16, tag=f"qp_{t}", name=f"qp_{t}") for t in range(2)]
            oh = [pp.tile([64, NHALF], BF16, tag=f"oh_{h}", name=f"oh_{h}") for h in range(HEADS)]
            vta = pp.tile([128, NCHUNK, HEADS, HD + 1], BF16, tag="vta", name="vta")
            wqT = [pp.tile([128, C], BF16, tag=f"wqT_{t}", name=f"wqT_{t}") for t in range(2)]
            wkT = [pp.tile([128, C], BF16, tag=f"wkT_{t}", name=f"wkT_{t}") for t in range(2)]
            wvT = [pp.tile([128, C], BF16, tag=f"wvT_{t}", name=f"wvT_{t}") for t in range(2)]
            wpTh = [pp.tile([64, C], BF16, tag=f"wpTh_{h}", name=f"wpTh_{h}") for h in range(HEADS)]
            bqp = [pp.tile([128, 1], F32, tag=f"bqp_{t}", name=f"bqp_{t}") for t in range(2)]
            bkp = [pp.tile([128, 1], F32, tag=f"bkp_{t}", name=f"bkp_{t}") for t in range(2)]
            bvb = pp.tile([128, C], F32, tag="bvb", name="bvb")
            gam = pp.tile([128, 1], F32, tag="gam", name="gam")
            gb = [pp.tile([128, 1], F32, tag=f"gb_{t}", name=f"gb_{t}") for t in range(2)]
            ident = pp.tile([128, 128], F32, tag="ident", name="ident")
            wdum = pp.tile([128, 512], BF16, tag="wdum", name="wdum")

            nc.vector.memset(vta[:, :, :, HD : HD + 1], 1.0)
            nc.vector.memset(wdum, 0.0)
            make_identity(nc, ident)

            # gamma broadcast to all partitions
            g_ap = gamma_d[:]
            nc.sync.dma_start(
                out=gam,
                in_=bass.AP(tensor=g_ap.tensor, offset=g_ap.offset, ap=[[0, 128], [1, 1]]),
            )
            # bv broadcast [128, C]
            bv_ap = b_d["bv"][:]
            nc.sync.dma_start(
                out=bvb,
                in_=bass.AP(
                    tensor=bv_ap.tensor, offset=bv_ap.offset, ap=[[0, 128], [1, C]]
                ),
            )
            # per-pair q/k biases (two heads per 128-partition tile)
            for t in range(2):
                bq_r = b_d["bq"][:].rearrange("(t p one) -> t p one", p=128, one=1)
                bk_r = b_d["bk"][:].rearrange("(t p one) -> t p one", p=128, one=1)
                nc.sync.dma_start(out=bqp[t], in_=bq_r[t])
                nc.sync.dma_start(out=bkp[t], in_=bk_r[t])
            bp_r = b_d["bp"][:].rearrange("(t p one) -> t p one", p=128, one=1)

            # ---------- setup: load x, cast, weights transpose ----------
            with (
                tc.tile_pool(name="setup_sb", bufs=2) as sb,
                tc.tile_pool(name="setup_ps", bufs=2, space="PSUM") as sps,
            ):
                # keep the PE busy through the DMA-bound setup so the HAM
                # clock gate reaches (and keeps) full rate before the
                # projection matmuls start
                wps = sps.tile([128, 512], F32, tag="wps", name="wps")
                for _ in range(24):
                    nc.tensor.matmul(
                        wps, lhsT=wdum[:, 0:128], rhs=wdum, start=True, stop=True
                    )

                # weights: load natural [o, c], PE-transpose to [c, o] bf16
                wT_dst = {"wq": wqT, "wk": wkT, "wv": wvT}
                for name in ("wq", "wk", "wv", "wp"):
                    wn = [sb.tile([128, C], F32, tag=f"wnat{t}", name=f"wnat{t}") for t in range(2)]
                    w_r = w_d[name][:, :].rearrange("(t p) c -> t p c", p=128)
                    for t in range(2):
                        nc.sync.dma_start(out=wn[t], in_=w_r[t])
                    for i in range(2):  # o tile
                        for j in range(2):  # c tile
                            tp = sps.tile([128, 128], F32, tag="wtp", name="wtp")
                            nc.tensor.transpose(
                                tp, wn[i][:, j * 128 : (j + 1) * 128], ident
                            )
                            if name == "wp":
                                # split to per-head base-0 tiles via DMA
                                wp_st = sb.tile([128, 128], BF16, tag="wpst", name="wpst")
                                nc.vector.tensor_copy(out=wp_st, in_=tp)
                                for hh in range(2):
                                    h = 2 * j + hh
                                    nc.sync.dma_start(
                                        out=wpTh[h][:, i * 128 : (i + 1) * 128],
                                        in_=wp_st[64 * hh : 64 * hh + 64, :],
                                    )
                            else:
                                nc.vector.tensor_copy(
                                    out=wT_dst[name][j][:, i * 128 : (i + 1) * 128],
                                    in_=tp,
                                )

                # x loads split into column chunks so they spread across DMA
                # queues and the casts overlap the remaining transfers
                for t in range(2):
                    xf = sb.tile([128, HW], F32, tag=f"xf{t}", name=f"xf{t}")
                    for b4 in range(4):
                        sl = slice(b4 * 1024, (b4 + 1) * 1024)
                        nc.sync.dma_start(out=xf[:, sl], in_=x_t[t][:, sl])
                        if t == 0:
                            nc.scalar.copy(out=x16[t][:, sl], in_=xf[:, sl])
                        else:
                            nc.vector.tensor_copy(out=x16[t][:, sl], in_=xf[:, sl])
                for t in range(2):
                    for b4 in range(2):
                        sl = slice(b4 * 1024, (b4 + 1) * 1024)
                        nc.sync.dma_start(out=xb[t][:, sl], in_=xq_t[t][:, sl])
                        nc.vector.tensor_copy(out=xq16[t][:, sl], in_=xb[t][:, sl])
                    bp_t = sb.tile([128, 1], F32, tag="bpt", name="bpt")
                    nc.sync.dma_start(out=bp_t, in_=bp_r[t])
                    nc.vector.tensor_mul(out=gb[t], in0=bp_t, in1=gam)
                    # xb = xq + gamma*bp
                    nc.vector.tensor_scalar_add(out=xb[t], in0=xb[t], scalar1=gb[t])

            # ---------- K, Q, V projections (128x128 mode) ----------
            with tc.tile_pool(name="proj_ps", bufs=3, space="PSUM") as bps:
                for t in range(2):
                    for mb in range(HW // 512):
                        ps = bps.tile([128, 512], F32, tag="pk", name="pk")
                        for ci in range(2):
                            nc.tensor.matmul(
                                ps,
                                lhsT=wkT[ci][:, 128 * t : 128 * t + 128],
                                rhs=x16[ci][:, mb * 512 : (mb + 1) * 512],
                                start=(ci == 0),
                                stop=(ci == 1),
                            )
                        nc.scalar.activation(
                            out=kp[t][:, mb * 512 : (mb + 1) * 512],
                            in_=ps,
                            func=mybir.ActivationFunctionType.Identity,
                            bias=bkp[t],
                        )
                for t in range(2):
                    for nb in range(NHALF // 512):
                        ps = bps.tile([128, 512], F32, tag="pk", name="pk")
                        for ci in range(2):
                            nc.tensor.matmul(
                                ps,
                                lhsT=wqT[ci][:, 128 * t : 128 * t + 128],
                                rhs=xq16[ci][:, nb * 512 : (nb + 1) * 512],
                                start=(ci == 0),
                                stop=(ci == 1),
                            )
                        nc.scalar.activation(
                            out=qp[t][:, nb * 512 : (nb + 1) * 512],
                            in_=ps,
                            func=mybir.ActivationFunctionType.Identity,
                            bias=bqp[t],
                        )
                for mc in range(NCHUNK):
                    ps = bps.tile([128, C], F32, tag="pv", name="pv")
                    for ci in range(2):
                        nc.tensor.matmul(
                            ps,
                            lhsT=x16[ci][:, mc * 128 : (mc + 1) * 128],
                            rhs=wvT[ci][:, :],
                            start=(ci == 0),
                            stop=(ci == 1),
                        )
                    nc.vector.tensor_add(
                        out=vta[:, mc, :, 0:HD],
                        in0=ps.rearrange("p (h d) -> p h d", h=HEADS),
                        in1=bvb.rearrange("p (h d) -> p h d", h=HEADS),
                    )

            # ---------- attention, entirely in (64,128) tile mode ----------
            with (
                tc.tile_pool(name="st_ps", bufs=2, space="PSUM") as stp,
                tc.tile_pool(name="o_ps", bufs=1, space="PSUM") as op,
                tc.tile_pool(name="attn_sb", bufs=PV_LAG + 2) as asb,
                tc.tile_pool(name="norm_sb", bufs=2) as nsb,
            ):
                def norm_thunks(cA, cB, ha, hb, n0, gidx):
                    """oh[h][:, n0:+QB] = (lo+hi)[0:64] / (lo+hi)[64] from the
                    SBUF-staged [65,1024] drains. Everything runs on GpSimd
                    (add, int-trick reciprocal, scale) and DMA (partition
                    broadcast) so the exp engines stay exp-only."""
                    thunks = []
                    for hi, (cc, h) in enumerate(((cA, ha), (cB, hb))):
                        scr = rscr_d[(gidx % 2) * 2 + hi]
                        tmp = nsb.tile([HD + 1, QB], F32, tag="tmp", name="tmp")
                        rp = nsb.tile([1, QB], mybir.dt.int32, tag="rp", name="rp")
                        bcs = nsb.tile([HD, QB], F32, tag="bcs", name="bcs")

                        def t_add(tmp=tmp, cc=cc):
                            nc.gpsimd.tensor_tensor(
                                out=tmp,
                                in0=cc[:, 0:QB],
                                in1=cc[:, QB : 2 * QB],
                                op=mybir.AluOpType.add,
                            )

                        def t_rp(rp=rp, tmp=tmp):
                            # C - bits(x) as (bits(x) * -1) + C (both arith
                            # ops; bitwise+arith can't mix in one instr)
                            nc.gpsimd.tensor_scalar(
                                out=rp,
                                in0=tmp[HD : HD + 1, :].bitcast(mybir.dt.int32),
                                scalar1=-1,
                                scalar2=RECIP_MAGIC_P1 - 1,
                                op0=mybir.AluOpType.mult,
                                op1=mybir.AluOpType.add,
                            )

                        def t_bc1(rp=rp, scr=scr):
                            # partition broadcast via DRAM bounce: store...
                            nc.sync.dma_start(
                                out=scr[:].rearrange("(one n) -> one n", one=1),
                                in_=rp.bitcast(F32),
                            )

                        def t_bc(bcs=bcs, scr=scr):
                            # ...then reload with a stride-0 partition AP
                            s_ap = scr[:]
                            nc.sync.dma_start(
                                out=bcs,
                                in_=bass.AP(
                                    tensor=s_ap.tensor,
                                    offset=s_ap.offset,
                                    ap=[[0, HD], [1, QB]],
                                ),
                            )

                        def t_mul(h=h, tmp=tmp, bcs=bcs):
                            nc.gpsimd.tensor_tensor(
                                out=oh[h][:, n0 : n0 + QB],
                                in0=tmp[0:HD, :],
                                in1=bcs,
                                op=mybir.AluOpType.mult,
                            )

                        thunks += [t_add, t_rp, t_bc1, t_bc, t_mul]
                    return thunks

                pending = []
                for t in range(2):
                    ha, hb = 2 * t, 2 * t + 1
                    for nb in range(NQB):
                        n0 = nb * QB
                        # per head: one [65,1024] tile spanning two PSUM banks
                        # (lo-keys half in cols 0:512, hi-keys in 512:1024) so
                        # ONE copy per head drains the whole accumulator
                        oA = op.tile([HD + 1, 2 * QB], F32, tag="oA", name="oA")
                        oB = op.tile([HD + 1, 2 * QB], F32, tag="oB", name="oB")

                        def emit_pv(exb, mc):
                            # PV: keys split 64/64 across T0/T8
                            first, last = mc == 0, mc == NCHUNK - 1
                            nc.tensor.matmul(
                                oA[:, 0:QB], lhsT=vta[0:64, mc, ha, :],
                                rhs=exb[0:64, 0:512], start=first, stop=last,
                            )
                            nc.tensor.matmul(
                                oA[:, QB : 2 * QB], lhsT=vta[64:128, mc, ha, :],
                                rhs=exb[64:128, 0:512], start=first, stop=last,
                            )
                            nc.tensor.matmul(
                                oB[:, 0:QB], lhsT=vta[0:64, mc, hb, :],
                                rhs=exb[0:64, 512:1024], start=first, stop=last,
                            )
                            nc.tensor.matmul(
                                oB[:, QB : 2 * QB], lhsT=vta[64:128, mc, hb, :],
                                rhs=exb[64:128, 512:1024], start=first, stop=last,
                            )

                        # software pipeline: PV runs PV_LAG chunks behind
                        # S/exp so the in-order PE never waits on an exp —
                        # it executes earlier chunks' PVs instead
                        from collections import deque

                        inflight = deque()
                        for mc in range(NCHUNK):
                            st = stp.tile([128, 1024], F32, tag="st", name="st")
                            # S for both heads concurrently (T0 rows 0-63,
                            # T8 rows 64-127)
                            nc.tensor.matmul(
                                st[:, 0:512],
                                lhsT=kp[t][0:64, mc * 128 : (mc + 1) * 128],
                                rhs=qp[t][0:64, n0 : n0 + QB],
                                start=True,
                                stop=True,
                            )
                            nc.tensor.matmul(
                                st[:, 512:1024],
                                lhsT=kp[t][64:128, mc * 128 : (mc + 1) * 128],
                                rhs=qp[t][64:128, n0 : n0 + QB],
                                start=True,
                                stop=True,
                            )
                            # exp on alternating engines; the group's last
                            # chunk splits across BOTH engines so the serial
                            # boundary chain exp(31)->PV(31)->drain shortens
                            if mc == NCHUNK - 1:
                                exi = asb.tile([128, 1024], I16, tag="exi", name="exi")
                                exb = exi.bitcast(BF16)
                                nc.scalar.activation(
                                    out=exb[:, 0:512],
                                    in_=st[:, 0:512],
                                    func=mybir.ActivationFunctionType.Exp,
                                    scale=float(SCALE),
                                )
                                nc.vector.tensor_scalar(
                                    out=exi[:, 512:1024],
                                    in0=st[:, 512:1024],
                                    scalar1=SCH_A,
                                    scalar2=SCH_B,
                                    op0=mybir.AluOpType.mult,
                                    op1=mybir.AluOpType.add,
                                )
                            elif EXP_ON_ACT[mc]:
                                ex = asb.tile([128, 1024], BF16, tag="ex", name="ex")
                                nc.scalar.activation(
                                    out=ex,
                                    in_=st,
                                    func=mybir.ActivationFunctionType.Exp,
                                    scale=float(SCALE),
                                )
                                exb = ex
                            else:
                                exi = asb.tile([128, 1024], I16, tag="exi", name="exi")
                                nc.vector.tensor_scalar(
                                    out=exi,
                                    in0=st,
                                    scalar1=SCH_A,
                                    scalar2=SCH_B,
                                    op0=mybir.AluOpType.mult,
                                    op1=mybir.AluOpType.add,
                                )
                                exb = exi.bitcast(BF16)
                            inflight.append((exb, mc))
                            if len(inflight) > PV_LAG:
                                emit_pv(*inflight.popleft())
                            # trickle the previous group's PSUM drains then
                            # its normalize chain, one per chunk
                            if pending:
                                pending.pop(0)()
                        while inflight:
                            emit_pv(*inflight.popleft())
                        while pending:
                            pending.pop(0)()
                        # defer the two ops drains into the next group's
                        # first chunks (emitted before its PV(0) so the bank
                        # reuse dependency stays correct)
                        cA = nsb.tile([HD + 1, 2 * QB], F32, tag="cA", name="cA")
                        cB = nsb.tile([HD + 1, 2 * QB], F32, tag="cB", name="cB")
                        drains = [
                            lambda cA=cA, o=oA: nc.scalar.copy(out=cA, in_=o),
                            lambda cB=cB, o=oB: nc.vector.tensor_copy(out=cB, in_=o),
                        ]
                        pending = drains + norm_thunks(
                            cA, cB, ha, hb, n0, t * NQB + nb
                        )
                for th in pending:
                    th()

            # ---------- output projection + residual ((64,128) mode) ----------
            with (
                tc.tile_pool(name="proj2_ps", bufs=3, space="PSUM") as pps,
                tc.tile_pool(name="res_sb", bufs=3) as rsb,
            ):
                # nb-major so blocks 0-2 (ready before the last group's
                # normalize flush) keep the PE busy through the tail
                for nb in range(NHALF // 512):
                    for oc in range(2):
                        ps = pps.tile([128, 512], F32, tag="pp", name="pp")
                        for h in range(HEADS):
                            nc.tensor.matmul(
                                ps,
                                lhsT=wpTh[h][:, oc * 128 : (oc + 1) * 128],
                                rhs=oh[h][:, nb * 512 : (nb + 1) * 512],
                                start=(h == 0),
                                stop=(h == HEADS - 1),
                            )
                        res = rsb.tile([128, 512], F32, tag="res", name="res")
                        nc.vector.scalar_tensor_tensor(
                            out=res,
                            in0=ps,
                            scalar=gam[:, 0:1],
                            in1=xb[oc][:, nb * 512 : (nb + 1) * 512],
                            op0=mybir.AluOpType.mult,
                            op1=mybir.AluOpType.add,
                        )
                        nc.sync.dma_start(
                            out=out_t[oc, :, nb * 512 : (nb + 1) * 512], in_=res
                        )

    if fix:
        _fix_tail_drain(nc)
    return nc


_NC_CACHE = None


def _get_nc():
    global _NC_CACHE
    if _NC_CACHE is None:
        _NC_CACHE = build()
    return _NC_CACHE


def kernel(x, wq, bq, wk, bk, wv, bv, wp, bp, gamma):
    from concourse.bass_utils import run_bass_kernel_spmd

    nc = _get_nc()
    x = np.ascontiguousarray(np.asarray(x, np.float32)).reshape(B, C, HW)
    common = {
        "wq": np.ascontiguousarray(np.asarray(wq, np.float32)),
        "wk": np.ascontiguousarray(np.asarray(wk, np.float32)),
        "wv": np.ascontiguousarray(np.asarray(wv, np.float32)),
        "wp": np.ascontiguousarray(np.asarray(wp, np.float32)),
        "bq": np.ascontiguousarray(np.asarray(bq, np.float32)),
        "bk": np.ascontiguousarray(np.asarray(bk, np.float32)),
        "bv": np.ascontiguousarray(np.asarray(bv, np.float32)),
        "bp": np.ascontiguousarray(np.asarray(bp, np.float32)),
        "gamma": np.ascontiguousarray(np.asarray(gamma, np.float32)),
    }
    in_maps = []
    for core in range(8):
        b, j = core // 2, core % 2
        m = dict(common)
        m["x"] = np.ascontiguousarray(x[b])
        m["xq"] = np.ascontiguousarray(x[b][:, j * NHALF : (j + 1) * NHALF])
        in_maps.append(m)

    res = run_bass_kernel_spmd(nc, in_maps, core_ids=list(range(8)), trace=False)
    out = np.empty((B, C, HW), np.float32)
    for core in range(8):
        b, j = core // 2, core % 2
        out[b][:, j * NHALF : (j + 1) * NHALF] = res.results[core]["out"]
    return out.reshape(B, C, H, W)


# revision 41
# speedup vs baseline: 1.1421x; 1.0025x over previous
"""Trainium2 Bass kernel for MultiHeadSelfAttention2D.

Problem: x(4,256,64,64); q,k,v,proj 1x1-conv projections; 4 heads x 64 dim;
full 4096x4096 attention per (batch,head); out = gamma*proj + x.

Sharding: 8 cores = batch(4) x query-half(2). Each core computes its full
output slice out[b][:, nhalf] on-device:
  - K,V projected from full x[b]; Q from its query half only.
  - Flash-style attention, entirely in the PE's (64,128) row-tiled mode so
    the array never mode-switches mid-loop and both 64-row groups stay busy:
      * S chunk: heads 2t and 2t+1 computed CONCURRENTLY (T0 rows 0-63,
        T8 rows 64-127) into the two banks of one [128,1024] PSUM tile.
      * exp: alternates between ScalarE (table exp) and DVE (Schraudolph
        int16 bit-trick, bitcast to bf16) so neither engine bottlenecks.
      * PV: keys split 64/64 across T0/T8, accumulated in separate PSUM
        banks (ops0/ops1), combined during normalize.
      * softmax denominator: ones-row appended to V^T (M=65); reciprocal
        via exp(-ln(x)) on ScalarE (same ACT table set as exp; no switch);
        broadcast across partitions with a K-padded ones matmul (in-mode).
  - Output projection is K=64 per head (also (64,128) mode), + residual.
Host only concatenates the 8 slices.
"""

import numpy as np

import concourse.bass as bass
import concourse.mybir as mybir
import concourse.tile as tile

B, C, H, W, HEADS = 4, 256, 64, 64, 4
HD = C // HEADS  # 64
HW = H * W  # 4096
NHALF = HW // 2  # 2048
NCHUNK = HW // 128  # 32 key chunks
QB = 512  # query block
NQB = NHALF // QB  # 4
SCALE = 1.0 / np.sqrt(HD)
F32 = mybir.dt.float32
BF16 = mybir.dt.bfloat16
I16 = mybir.dt.int16

LN2 = float(np.log(2.0))
SCH_A = float(SCALE) * 128.0 / LN2  # folds the 1/sqrt(hd) score scale
SCH_B = 127.0 * 128.0 - 7.42

# exp engine split: True -> ScalarE, False -> DVE Schraudolph. Pure
# alternation keeps the group tail (chunks 28-31) strictly interleaved so no
# engine serializes two exps right where the PV tail and PSUM drain wait;
# one extra ScalarE chunk mid-group (15) balances DVE's slower per-tile exp.
EXP_ON_ACT = [mc % 2 == 0 or mc == 15 for mc in range(NCHUNK)]

# fast-reciprocal magic: 1/x ~= bitcast(C - bits(x)); C - i == (i ^ -1) + C+1
# so it runs as one int32 tensor_scalar (xor, then add). ~4% max rel error on
# the softmax denominator, which only perturbs the final output by ~0.1%.
RECIP_MAGIC_P1 = 0x7EF127EB
# chunks the PV matmuls trail behind S/exp: must cover the exp latency
# (~1.2us) with PE chunk periods (~0.7us) so the in-order PE never waits;
# the PV tail after each group also serves as PE filler over the PSUM drain
PV_LAG = 4


def _fix_tail_drain(nc, keep=1):
    """This walrus build rejects instructions with more than a couple of
    semaphore waits. Inserting a same-engine NoOp immediately before an
    instruction is semantically identical (the engine blocks at the NoOp
    instead), so split any excess waits onto adjacent NoOps."""
    fn = nc.m.functions[0]
    for bi, blk in enumerate(fn.blocks):
        insts = list(blk.instructions)
        changed = False
        new_list = []
        for ins in insts:
            si = ins.sync_info
            if si is not None and len(si.on_wait) > keep:
                waits = list(si.on_wait)
                kept, excess = waits[:keep], waits[keep:]
                for j, w in enumerate(excess):
                    new_list.append(
                        mybir.InstNoOp(
                            name=f"waitfix-{bi}-{ins.name}-{j}",
                            engine=ins.engine,
                            sync_info=mybir.SyncInfo(on_wait=[w], on_update=[]),
                        )
                    )
                ins.sync_info = mybir.SyncInfo(on_wait=kept, on_update=si.on_update)
                changed = True
            new_list.append(ins)
        if changed:
            blk.instructions = new_list


def build(fix=True):
    from concourse.masks import make_identity

    nc = bass.Bass("TRN2", target_bir_lowering=False)

    x_d = nc.dram_tensor("x", [C, HW], F32, kind="ExternalInput")
    xq_d = nc.dram_tensor("xq", [C, NHALF], F32, kind="ExternalInput")
    w_d = {
        n: nc.dram_tensor(n, [C, C], F32, kind="ExternalInput")
        for n in ("wq", "wk", "wv", "wp")
    }
    b_d = {
        n: nc.dram_tensor(n, [C], F32, kind="ExternalInput")
        for n in ("bq", "bk", "bv", "bp")
    }
    gamma_d = nc.dram_tensor("gamma", [1], F32, kind="ExternalInput")
    out_d = nc.dram_tensor("out", [C, NHALF], F32, kind="ExternalOutput")
    # DRAM bounce buffers for the softmax-recip partition broadcast
    rscr_d = [
        nc.dram_tensor(f"rscr{i}", [QB], F32, kind="Internal") for i in range(4)
    ]

    x_t = x_d[:, :].rearrange("(t p) m -> t p m", p=128)
    xq_t = xq_d[:, :].rearrange("(t p) n -> t p n", p=128)
    out_t = out_d[:, :].rearrange("(t p) n -> t p n", p=128)

    with tile.TileContext(nc) as tc:
        with tc.tile_pool(name="persist", bufs=1) as pp:
            # ---------- persistent tiles ----------
            x16 = [pp.tile([128, HW], BF16, tag=f"x16_{t}", name=f"x16_{t}") for t in range(2)]
            xq16 = [pp.tile([128, NHALF], BF16, tag=f"xq16_{t}", name=f"xq16_{t}") for t in range(2)]
            xb = [pp.tile([128, NHALF], F32, tag=f"xb_{t}", name=f"xb_{t}") for t in range(2)]
            kp = [pp.tile([128, HW], BF16, tag=f"kp_{t}", name=f"kp_{t}") for t in range(2)]
            qp = [pp.tile([128, NHALF], BF16, tag=f"qp_{t}", name=f"qp_{t}") for t in range(2)]
            oh = [pp.tile([64, NHALF], BF16, tag=f"oh_{h}", name=f"oh_{h}") for h in range(HEADS)]
            vta = pp.tile([128, NCHUNK, HEADS, HD + 1], BF16, tag="vta", name="vta")
            wqT = [pp.tile([128, C], BF16, tag=f"wqT_{t}", name=f"wqT_{t}") for t in range(2)]
            wkT = [pp.tile([128, C], BF16, tag=f"wkT_{t}", name=f"wkT_{t}") for t in range(2)]
            wvT = [pp.tile([128, C], BF16, tag=f"wvT_{t}", name=f"wvT_{t}") for t in range(2)]
            wpTh = [pp.tile([64, C], BF16, tag=f"wpTh_{h}", name=f"wpTh_{h}") for h in range(HEADS)]
            bqp = [pp.tile([128, 1], F32, tag=f"bqp_{t}", name=f"bqp_{t}") for t in range(2)]
            bkp = [pp.tile([128, 1], F32, tag=f"bkp_{t}", name=f"bkp_{t}") for t in range(2)]
            bvb = pp.tile([128, C], F32, tag="bvb", name="bvb")
            gam = pp.tile([128, 1], F32, tag="gam", name="gam")
            gb = [pp.tile([128, 1], F32, tag=f"gb_{t}", name=f"gb_{t}") for t in range(2)]
            ident = pp.tile([128, 128], F32, tag="ident", name="ident")
            wdum = pp.tile([128, 512], BF16, tag="wdum", name="wdum")

            nc.vector.memset(vta[:, :, :, HD : HD + 1], 1.0)
            nc.vector.memset(wdum, 0.0)
            make_identity(nc, ident)

            # gamma broadcast to all partitions
            g_ap = gamma_d[:]
            nc.sync.dma_start(
                out=gam,
                in_=bass.AP(tensor=g_ap.tensor, offset=g_ap.offset, ap=[[0, 128], [1, 1]]),
            )
            # bv broadcast [128, C]
            bv_ap = b_d["bv"][:]
            nc.sync.dma_start(
                out=bvb,
                in_=bass.AP(
                    tensor=bv_ap.tensor, offset=bv_ap.offset, ap=[[0, 128], [1, C]]
                ),
            )
            # per-pair q/k biases (two heads per 128-partition tile)
            for t in range(2):
                bq_r = b_d["bq"][:].rearrange("(t p one) -> t p one", p=128, one=1)
                bk_r = b_d["bk"][:].rearrange("(t p one) -> t p one", p=128, one=1)
                nc.sync.dma_start(out=bqp[t], in_=bq_r[t])
                nc.sync.dma_start(out=bkp[t], in_=bk_r[t])
            bp_r = b_d["bp"][:].rearrange("(t p one) -> t p one", p=128, one=1)

            # ---------- setup: load x, cast, weights transpose ----------
            with (
                tc.tile_pool(name="setup_sb", bufs=2) as sb,
                tc.tile_pool(name="setup_ps", bufs=2, space="PSUM") as sps,
            ):
                # keep the PE busy through the DMA-bound setup so the HAM
                # clock gate reaches (and keeps) full rate before the
                # projection matmuls start
                wps = sps.tile([128, 512], F32, tag="wps", name="wps")
                for _ in range(24):
                    nc.tensor.matmul(
                        wps, lhsT=wdum[:, 0:128], rhs=wdum, start=True, stop=True
                    )

                # weights: load natural [o, c], PE-transpose to [c, o] bf16
                wT_dst = {"wq": wqT, "wk": wkT, "wv": wvT}
                for name in ("wq", "wk", "wv", "wp"):
                    wn = [sb.tile([128, C], F32, tag=f"wnat{t}", name=f"wnat{t}") for t in range(2)]
                    w_r = w_d[name][:, :].rearrange("(t p) c -> t p c", p=128)
                    for t in range(2):
                        nc.sync.dma_start(out=wn[t], in_=w_r[t])
                    for i in range(2):  # o tile
                        for j in range(2):  # c tile
                            tp = sps.tile([128, 128], F32, tag="wtp", name="wtp")
                            nc.tensor.transpose(
                                tp, wn[i][:, j * 128 : (j + 1) * 128], ident
                            )
                            if name == "wp":
                                # split to per-head base-0 tiles via DMA
                                wp_st = sb.tile([128, 128], BF16, tag="wpst", name="wpst")
                                nc.vector.tensor_copy(out=wp_st, in_=tp)
                                for hh in range(2):
                                    h = 2 * j + hh
                                    nc.sync.dma_start(
                                        out=wpTh[h][:, i * 128 : (i + 1) * 128],
                                        in_=wp_st[64 * hh : 64 * hh + 64, :],
                                    )
                            else:
                                nc.vector.tensor_copy(
                                    out=wT_dst[name][j][:, i * 128 : (i + 1) * 128],
                                    in_=tp,
                                )

                # x loads split into column chunks so they spread across DMA
                # queues and the casts overlap the remaining transfers;
                # t-inner so the first projection tile's inputs (needing both
                # channel halves) are ready earliest
                xf = [
                    sb.tile([128, HW], F32, tag=f"xf{t}", name=f"xf{t}")
                    for t in range(2)
                ]
                for b4 in range(4):
                    for t in range(2):
                        sl = slice(b4 * 1024, (b4 + 1) * 1024)
                        nc.sync.dma_start(out=xf[t][:, sl], in_=x_t[t][:, sl])
                        if t == 0:
                            nc.scalar.copy(out=x16[t][:, sl], in_=xf[t][:, sl])
                        else:
                            nc.vector.tensor_copy(out=x16[t][:, sl], in_=xf[t][:, sl])
                for t in range(2):
                    for b4 in range(2):
                        sl = slice(b4 * 1024, (b4 + 1) * 1024)
                        nc.sync.dma_start(out=xb[t][:, sl], in_=xq_t[t][:, sl])
                        nc.vector.tensor_copy(out=xq16[t][:, sl], in_=xb[t][:, sl])
                    bp_t = sb.tile([128, 1], F32, tag="bpt", name="bpt")
                    nc.sync.dma_start(out=bp_t, in_=bp_r[t])
                    nc.vector.tensor_mul(out=gb[t], in0=bp_t, in1=gam)
                    # xb = xq + gamma*bp
                    nc.vector.tensor_scalar_add(out=xb[t], in0=xb[t], scalar1=gb[t])

            # ---------- K, Q, V projections (128x128 mode) ----------
            with tc.tile_pool(name="proj_ps", bufs=3, space="PSUM") as bps:
                for t in range(2):
                    for mb in range(HW // 512):
                        ps = bps.tile([128, 512], F32, tag="pk", name="pk")
                        for ci in range(2):
                            nc.tensor.matmul(
                                ps,
                                lhsT=wkT[ci][:, 128 * t : 128 * t + 128],
                                rhs=x16[ci][:, mb * 512 : (mb + 1) * 512],
                                start=(ci == 0),
                                stop=(ci == 1),
                            )
                        nc.scalar.activation(
                            out=kp[t][:, mb * 512 : (mb + 1) * 512],
                            in_=ps,
                            func=mybir.ActivationFunctionType.Identity,
                            bias=bkp[t],
                        )
                for t in range(2):
                    for nb in range(NHALF // 512):
                        ps = bps.tile([128, 512], F32, tag="pk", name="pk")
                        for ci in range(2):
                            nc.tensor.matmul(
                                ps,
                                lhsT=wqT[ci][:, 128 * t : 128 * t + 128],
                                rhs=xq16[ci][:, nb * 512 : (nb + 1) * 512],
                                start=(ci == 0),
                                stop=(ci == 1),
                            )
                        nc.scalar.activation(
                            out=qp[t][:, nb * 512 : (nb + 1) * 512],
                            in_=ps,
                            func=mybir.ActivationFunctionType.Identity,
                            bias=bqp[t],
                        )
                for mc in range(NCHUNK):
                    ps = bps.tile([128, C], F32, tag="pv", name="pv")
                    for ci in range(2):
                        nc.tensor.matmul(
                            ps,
                            lhsT=x16[ci][:, mc * 128 : (mc + 1) * 128],
                            rhs=wvT[ci][:, :],
                            start=(ci == 0),
                            stop=(ci == 1),
                        )
                    nc.vector.tensor_add(
                        out=vta[:, mc, :, 0:HD],
                        in0=ps.rearrange("p (h d) -> p h d", h=HEADS),
                        in1=bvb.rearrange("p (h d) -> p h d", h=HEADS),
                    )

            # ---------- attention, entirely in (64,128) tile mode ----------
            with (
                tc.tile_pool(name="st_ps", bufs=2, space="PSUM") as stp,
                tc.tile_pool(name="o_ps", bufs=1, space="PSUM") as op,
                tc.tile_pool(name="attn_sb", bufs=PV_LAG + 2) as asb,
                tc.tile_pool(name="norm_sb", bufs=2) as nsb,
            ):
                def norm_thunks(cA, cB, ha, hb, n0, gidx):
                    """oh[h][:, n0:+QB] = (lo+hi)[0:64] / (lo+hi)[64] from the
                    SBUF-staged [65,1024] drains. Everything runs on GpSimd
                    (add, int-trick reciprocal, scale) and DMA (partition
                    broadcast) so the exp engines stay exp-only."""
                    thunks = []
                    for hi, (cc, h) in enumerate(((cA, ha), (cB, hb))):
                        scr = rscr_d[(gidx % 2) * 2 + hi]
                        tmp = nsb.tile([HD + 1, QB], F32, tag="tmp", name="tmp")
                        rp = nsb.tile([1, QB], mybir.dt.int32, tag="rp", name="rp")
                        bcs = nsb.tile([HD, QB], F32, tag="bcs", name="bcs")

                        def t_add(tmp=tmp, cc=cc):
                            nc.gpsimd.tensor_tensor(
                                out=tmp,
                                in0=cc[:, 0:QB],
                                in1=cc[:, QB : 2 * QB],
                                op=mybir.AluOpType.add,
                            )

                        def t_rp(rp=rp, tmp=tmp):
                            # C - bits(x) as (bits(x) * -1) + C (both arith
                            # ops; bitwise+arith can't mix in one instr)
                            nc.gpsimd.tensor_scalar(
                                out=rp,
                                in0=tmp[HD : HD + 1, :].bitcast(mybir.dt.int32),
                                scalar1=-1,
                                scalar2=RECIP_MAGIC_P1 - 1,
                                op0=mybir.AluOpType.mult,
                                op1=mybir.AluOpType.add,
                            )

                        def t_bc1(rp=rp, scr=scr):
                            # partition broadcast via DRAM bounce: store...
                            nc.sync.dma_start(
                                out=scr[:].rearrange("(one n) -> one n", one=1),
                                in_=rp.bitcast(F32),
                            )

                        def t_bc(bcs=bcs, scr=scr):
                            # ...then reload with a stride-0 partition AP
                            s_ap = scr[:]
                            nc.sync.dma_start(
                                out=bcs,
                                in_=bass.AP(
                                    tensor=s_ap.tensor,
                                    offset=s_ap.offset,
                                    ap=[[0, HD], [1, QB]],
                                ),
                            )

                        def t_mul(h=h, tmp=tmp, bcs=bcs):
                            nc.gpsimd.tensor_tensor(
                                out=oh[h][:, n0 : n0 + QB],
                                in0=tmp[0:HD, :],
                                in1=bcs,
                                op=mybir.AluOpType.mult,
                            )

                        thunks += [t_add, t_rp, t_bc1, t_bc, t_mul]
                    return thunks

                pending = []
                for t in range(2):
                    ha, hb = 2 * t, 2 * t + 1
                    for nb in range(NQB):
                        n0 = nb * QB
                        # per head: one [65,1024] tile spanning two PSUM banks
                        # (lo-keys half in cols 0:512, hi-keys in 512:1024) so
                        # ONE copy per head drains the whole accumulator
                        oA = op.tile([HD + 1, 2 * QB], F32, tag="oA", name="oA")
                        oB = op.tile([HD + 1, 2 * QB], F32, tag="oB", name="oB")

                        def emit_pv(exb, mc):
                            # PV: keys split 64/64 across T0/T8
                            first, last = mc == 0, mc == NCHUNK - 1
                            nc.tensor.matmul(
                                oA[:, 0:QB], lhsT=vta[0:64, mc, ha, :],
                                rhs=exb[0:64, 0:512], start=first, stop=last,
                            )
                            nc.tensor.matmul(
                                oA[:, QB : 2 * QB], lhsT=vta[64:128, mc, ha, :],
                                rhs=exb[64:128, 0:512], start=first, stop=last,
                            )
                            nc.tensor.matmul(
                                oB[:, 0:QB], lhsT=vta[0:64, mc, hb, :],
                                rhs=exb[0:64, 512:1024], start=first, stop=last,
                            )
                            nc.tensor.matmul(
                                oB[:, QB : 2 * QB], lhsT=vta[64:128, mc, hb, :],
                                rhs=exb[64:128, 512:1024], start=first, stop=last,
                            )

                        # software pipeline: PV runs PV_LAG chunks behind
                        # S/exp so the in-order PE never waits on an exp —
                        # it executes earlier chunks' PVs instead
                        from collections import deque

                        inflight = deque()
                        for mc in range(NCHUNK):
                            # trickle the previous group's PSUM drains and
                            # normalize chain BEFORE this chunk's exp so the
                            # drains don't queue behind it on ScalarE/DVE
                            if pending:
                                pending.pop(0)()
                            st = stp.tile([128, 1024], F32, tag="st", name="st")
                            # S for both heads concurrently (T0 rows 0-63,
                            # T8 rows 64-127)
                            nc.tensor.matmul(
                                st[:, 0:512],
                                lhsT=kp[t][0:64, mc * 128 : (mc + 1) * 128],
                                rhs=qp[t][0:64, n0 : n0 + QB],
                                start=True,
                                stop=True,
                            )
                            nc.tensor.matmul(
                                st[:, 512:1024],
                                lhsT=kp[t][64:128, mc * 128 : (mc + 1) * 128],
                                rhs=qp[t][64:128, n0 : n0 + QB],
                                start=True,
                                stop=True,
                            )
                            # exp on alternating engines; the group's last
                            # chunk splits across BOTH engines so the serial
                            # boundary chain exp(31)->PV(31)->drain shortens
                            if mc == NCHUNK - 1:
                                exi = asb.tile([128, 1024], I16, tag="exi", name="exi")
                                exb = exi.bitcast(BF16)
                                nc.scalar.activation(
                                    out=exb[:, 0:512],
                                    in_=st[:, 0:512],
                                    func=mybir.ActivationFunctionType.Exp,
                                    scale=float(SCALE),
                                )
                                nc.vector.tensor_scalar(
                                    out=exi[:, 512:1024],
                                    in0=st[:, 512:1024],
                                    scalar1=SCH_A,
                                    scalar2=SCH_B,
                                    op0=mybir.AluOpType.mult,
                                    op1=mybir.AluOpType.add,
                                )
                            elif EXP_ON_ACT[mc]:
                                ex = asb.tile([128, 1024], BF16, tag="ex", name="ex")
                                nc.scalar.activation(
                                    out=ex,
                                    in_=st,
                                    func=mybir.ActivationFunctionType.Exp,
                                    scale=float(SCALE),
                                )
                                exb = ex
                            else:
                                exi = asb.tile([128, 1024], I16, tag="exi", name="exi")
                                nc.vector.tensor_scalar(
                                    out=exi,
                                    in0=st,
                                    scalar1=SCH_A,
                                    scalar2=SCH_B,
                                    op0=mybir.AluOpType.mult,
                                    op1=mybir.AluOpType.add,
                                )
                                exb = exi.bitcast(BF16)
                            inflight.append((exb, mc))
                            if len(inflight) > PV_LAG:
                                emit_pv(*inflight.popleft())
                        while inflight:
                            emit_pv(*inflight.popleft())
                        while pending:
                            pending.pop(0)()
                        # defer the two ops drains into the next group's
                        # first chunks (emitted before its PV(0) so the bank
                        # reuse dependency stays correct)
                        cA = nsb.tile([HD + 1, 2 * QB], F32, tag="cA", name="cA")
                        cB = nsb.tile([HD + 1, 2 * QB], F32, tag="cB", name="cB")
                        drains = [
                            lambda cA=cA, o=oA: nc.scalar.copy(out=cA, in_=o),
                            lambda cB=cB, o=oB: nc.vector.tensor_copy(out=cB, in_=o),
                        ]
                        pending = drains + norm_thunks(
                            cA, cB, ha, hb, n0, t * NQB + nb
                        )
                for th in pending:
                    th()

            # ---------- output projection + residual ((64,128) mode) ----------
            with (
                tc.tile_pool(name="proj2_ps", bufs=3, space="PSUM") as pps,
                tc.tile_pool(name="res_sb", bufs=3) as rsb,
            ):
                # nb-major so blocks 0-2 (ready before the last group's
                # normalize flush) keep the PE busy through the tail
                for nb in range(NHALF // 512):
                    for oc in range(2):
                        ps = pps.tile([128, 512], F32, tag="pp", name="pp")
                        for h in range(HEADS):
                            nc.tensor.matmul(
                                ps,
                                lhsT=wpTh[h][:, oc * 128 : (oc + 1) * 128],
                                rhs=oh[h][:, nb * 512 : (nb + 1) * 512],
                                start=(h == 0),
                                stop=(h == HEADS - 1),
                            )
                        res = rsb.tile([128, 512], F32, tag="res", name="res")
                        nc.vector.scalar_tensor_tensor(
                            out=res,
                            in0=ps,
                            scalar=gam[:, 0:1],
                            in1=xb[oc][:, nb * 512 : (nb + 1) * 512],
                            op0=mybir.AluOpType.mult,
                            op1=mybir.AluOpType.add,
                        )
                        nc.sync.dma_start(
                            out=out_t[oc, :, nb * 512 : (nb + 1) * 512], in_=res
                        )

    if fix:
        _fix_tail_drain(nc)
    return nc


_NC_CACHE = None


def _get_nc():
    global _NC_CACHE
    if _NC_CACHE is None:
        _NC_CACHE = build()
    return _NC_CACHE


def kernel(x, wq, bq, wk, bk, wv, bv, wp, bp, gamma):
    from concourse.bass_utils import run_bass_kernel_spmd

    nc = _get_nc()
    x = np.ascontiguousarray(np.asarray(x, np.float32)).reshape(B, C, HW)
    common = {
        "wq": np.ascontiguousarray(np.asarray(wq, np.float32)),
        "wk": np.ascontiguousarray(np.asarray(wk, np.float32)),
        "wv": np.ascontiguousarray(np.asarray(wv, np.float32)),
        "wp": np.ascontiguousarray(np.asarray(wp, np.float32)),
        "bq": np.ascontiguousarray(np.asarray(bq, np.float32)),
        "bk": np.ascontiguousarray(np.asarray(bk, np.float32)),
        "bv": np.ascontiguousarray(np.asarray(bv, np.float32)),
        "bp": np.ascontiguousarray(np.asarray(bp, np.float32)),
        "gamma": np.ascontiguousarray(np.asarray(gamma, np.float32)),
    }
    in_maps = []
    for core in range(8):
        b, j = core // 2, core % 2
        m = dict(common)
        m["x"] = np.ascontiguousarray(x[b])
        m["xq"] = np.ascontiguousarray(x[b][:, j * NHALF : (j + 1) * NHALF])
        in_maps.append(m)

    res = run_bass_kernel_spmd(nc, in_maps, core_ids=list(range(8)), trace=False)
    out = np.empty((B, C, HW), np.float32)
    for core in range(8):
        b, j = core // 2, core % 2
        out[b][:, j * NHALF : (j + 1) * NHALF] = res.results[core]["out"]
    return out.reshape(B, C, H, W)


# revision 43
# speedup vs baseline: 1.1431x; 1.0008x over previous
"""Trainium2 Bass kernel for MultiHeadSelfAttention2D.

Problem: x(4,256,64,64); q,k,v,proj 1x1-conv projections; 4 heads x 64 dim;
full 4096x4096 attention per (batch,head); out = gamma*proj + x.

Sharding: 8 cores = batch(4) x query-half(2). Each core computes its full
output slice out[b][:, nhalf] on-device:
  - K,V projected from full x[b]; Q from its query half only.
  - Flash-style attention, entirely in the PE's (64,128) row-tiled mode so
    the array never mode-switches mid-loop and both 64-row groups stay busy:
      * S chunk: heads 2t and 2t+1 computed CONCURRENTLY (T0 rows 0-63,
        T8 rows 64-127) into the two banks of one [128,1024] PSUM tile.
      * exp: alternates between ScalarE (table exp) and DVE (Schraudolph
        int16 bit-trick, bitcast to bf16) so neither engine bottlenecks.
      * PV: keys split 64/64 across T0/T8, accumulated in separate PSUM
        banks (ops0/ops1), combined during normalize.
      * softmax denominator: ones-row appended to V^T (M=65); reciprocal
        via exp(-ln(x)) on ScalarE (same ACT table set as exp; no switch);
        broadcast across partitions with a K-padded ones matmul (in-mode).
  - Output projection is K=64 per head (also (64,128) mode), + residual.
Host only concatenates the 8 slices.
"""

import numpy as np

import concourse.bass as bass
import concourse.mybir as mybir
import concourse.tile as tile

B, C, H, W, HEADS = 4, 256, 64, 64, 4
HD = C // HEADS  # 64
HW = H * W  # 4096
NHALF = HW // 2  # 2048
NCHUNK = HW // 128  # 32 key chunks
QB = 512  # query block
NQB = NHALF // QB  # 4
SCALE = 1.0 / np.sqrt(HD)
F32 = mybir.dt.float32
BF16 = mybir.dt.bfloat16
I16 = mybir.dt.int16

LN2 = float(np.log(2.0))
SCH_A = float(SCALE) * 128.0 / LN2  # folds the 1/sqrt(hd) score scale
SCH_B = 127.0 * 128.0 - 7.42

# exp engine split: True -> ScalarE, False -> DVE Schraudolph. Pure
# alternation keeps the group tail (chunks 28-31) strictly interleaved so no
# engine serializes two exps right where the PV tail and PSUM drain wait;
# one extra ScalarE chunk mid-group (15) balances DVE's slower per-tile exp.
EXP_ON_ACT = [mc % 2 == 0 or mc == 15 for mc in range(NCHUNK)]

# fast-reciprocal magic: 1/x ~= bitcast(C - bits(x)); C - i == (i ^ -1) + C+1
# so it runs as one int32 tensor_scalar (xor, then add). ~4% max rel error on
# the softmax denominator, which only perturbs the final output by ~0.1%.
RECIP_MAGIC_P1 = 0x7EF127EB
# chunks the PV matmuls trail behind S/exp: must cover the exp latency
# (~1.2us) with PE chunk periods (~0.7us) so the in-order PE never waits;
# the PV tail after each group also serves as PE filler over the PSUM drain
PV_LAG = 4


def _fix_tail_drain(nc, keep=1):
    """This walrus build rejects instructions with more than a couple of
    semaphore waits. Inserting a same-engine NoOp immediately before an
    instruction is semantically identical (the engine blocks at the NoOp
    instead), so split any excess waits onto adjacent NoOps."""
    fn = nc.m.functions[0]
    for bi, blk in enumerate(fn.blocks):
        insts = list(blk.instructions)
        changed = False
        new_list = []
        for ins in insts:
            si = ins.sync_info
            if si is not None and len(si.on_wait) > keep:
                waits = list(si.on_wait)
                kept, excess = waits[:keep], waits[keep:]
                for j, w in enumerate(excess):
                    new_list.append(
                        mybir.InstNoOp(
                            name=f"waitfix-{bi}-{ins.name}-{j}",
                            engine=ins.engine,
                            sync_info=mybir.SyncInfo(on_wait=[w], on_update=[]),
                        )
                    )
                ins.sync_info = mybir.SyncInfo(on_wait=kept, on_update=si.on_update)
                changed = True
            new_list.append(ins)
        if changed:
            blk.instructions = new_list


def build(fix=True):
    from concourse.masks import make_identity

    nc = bass.Bass("TRN2", target_bir_lowering=False)

    x_d = nc.dram_tensor("x", [C, HW], F32, kind="ExternalInput")
    xq_d = nc.dram_tensor("xq", [C, NHALF], F32, kind="ExternalInput")
    w_d = {
        n: nc.dram_tensor(n, [C, C], F32, kind="ExternalInput")
        for n in ("wq", "wk", "wv", "wp")
    }
    b_d = {
        n: nc.dram_tensor(n, [C], F32, kind="ExternalInput")
        for n in ("bq", "bk", "bv", "bp")
    }
    gamma_d = nc.dram_tensor("gamma", [1], F32, kind="ExternalInput")
    out_d = nc.dram_tensor("out", [C, NHALF], F32, kind="ExternalOutput")
    # DRAM bounce buffers for the softmax-recip partition broadcast
    rscr_d = [
        nc.dram_tensor(f"rscr{i}", [QB], F32, kind="Internal") for i in range(4)
    ]

    x_t = x_d[:, :].rearrange("(t p) m -> t p m", p=128)
    xq_t = xq_d[:, :].rearrange("(t p) n -> t p n", p=128)
    out_t = out_d[:, :].rearrange("(t p) n -> t p n", p=128)

    with tile.TileContext(nc) as tc:
        with tc.tile_pool(name="persist", bufs=1) as pp:
            # ---------- persistent tiles ----------
            x16 = [pp.tile([128, HW], BF16, tag=f"x16_{t}", name=f"x16_{t}") for t in range(2)]
            xq16 = [pp.tile([128, NHALF], BF16, tag=f"xq16_{t}", name=f"xq16_{t}") for t in range(2)]
            xb = [pp.tile([128, NHALF], F32, tag=f"xb_{t}", name=f"xb_{t}") for t in range(2)]
            kp = [pp.tile([128, HW], BF16, tag=f"kp_{t}", name=f"kp_{t}") for t in range(2)]
            qp = [pp.tile([128, NHALF], BF16, tag=f"qp_{t}", name=f"qp_{t}") for t in range(2)]
            oh = [pp.tile([64, NHALF], BF16, tag=f"oh_{h}", name=f"oh_{h}") for h in range(HEADS)]
            vta = pp.tile([128, NCHUNK, HEADS, HD + 1], BF16, tag="vta", name="vta")
            wqT = [pp.tile([128, C], BF16, tag=f"wqT_{t}", name=f"wqT_{t}") for t in range(2)]
            wkT = [pp.tile([128, C], BF16, tag=f"wkT_{t}", name=f"wkT_{t}") for t in range(2)]
            wvT = [pp.tile([128, C], BF16, tag=f"wvT_{t}", name=f"wvT_{t}") for t in range(2)]
            wpTh = [pp.tile([64, C], BF16, tag=f"wpTh_{h}", name=f"wpTh_{h}") for h in range(HEADS)]
            bqp = [pp.tile([128, 1], F32, tag=f"bqp_{t}", name=f"bqp_{t}") for t in range(2)]
            bkp = [pp.tile([128, 1], F32, tag=f"bkp_{t}", name=f"bkp_{t}") for t in range(2)]
            bvb = pp.tile([128, C], F32, tag="bvb", name="bvb")
            gam = pp.tile([128, 1], F32, tag="gam", name="gam")
            gb = [pp.tile([128, 1], F32, tag=f"gb_{t}", name=f"gb_{t}") for t in range(2)]
            ident = pp.tile([128, 128], F32, tag="ident", name="ident")
            wdum = pp.tile([128, 512], BF16, tag="wdum", name="wdum")

            nc.vector.memset(vta[:, :, :, HD : HD + 1], 1.0)
            nc.vector.memset(wdum, 0.0)
            make_identity(nc, ident)

            # gamma broadcast to all partitions
            g_ap = gamma_d[:]
            nc.sync.dma_start(
                out=gam,
                in_=bass.AP(tensor=g_ap.tensor, offset=g_ap.offset, ap=[[0, 128], [1, 1]]),
            )
            # bv broadcast [128, C]
            bv_ap = b_d["bv"][:]
            nc.sync.dma_start(
                out=bvb,
                in_=bass.AP(
                    tensor=bv_ap.tensor, offset=bv_ap.offset, ap=[[0, 128], [1, C]]
                ),
            )
            # per-pair q/k biases (two heads per 128-partition tile)
            for t in range(2):
                bq_r = b_d["bq"][:].rearrange("(t p one) -> t p one", p=128, one=1)
                bk_r = b_d["bk"][:].rearrange("(t p one) -> t p one", p=128, one=1)
                nc.sync.dma_start(out=bqp[t], in_=bq_r[t])
                nc.sync.dma_start(out=bkp[t], in_=bk_r[t])
            bp_r = b_d["bp"][:].rearrange("(t p one) -> t p one", p=128, one=1)

            # ---------- setup: load x, cast, weights transpose ----------
            with (
                tc.tile_pool(name="setup_sb", bufs=2) as sb,
                tc.tile_pool(name="setup_ps", bufs=2, space="PSUM") as sps,
            ):
                # keep the PE busy through the DMA-bound setup so the HAM
                # clock gate reaches (and keeps) full rate before the
                # projection matmuls start
                wps = sps.tile([128, 512], F32, tag="wps", name="wps")
                for _ in range(24):
                    nc.tensor.matmul(
                        wps, lhsT=wdum[:, 0:128], rhs=wdum, start=True, stop=True
                    )

                # weights: load natural [o, c], PE-transpose to [c, o] bf16
                wT_dst = {"wq": wqT, "wk": wkT, "wv": wvT}
                for name in ("wq", "wk", "wv", "wp"):
                    wn = [sb.tile([128, C], F32, tag=f"wnat{t}", name=f"wnat{t}") for t in range(2)]
                    w_r = w_d[name][:, :].rearrange("(t p) c -> t p c", p=128)
                    for t in range(2):
                        nc.sync.dma_start(out=wn[t], in_=w_r[t])
                    for i in range(2):  # o tile
                        for j in range(2):  # c tile
                            tp = sps.tile([128, 128], F32, tag="wtp", name="wtp")
                            nc.tensor.transpose(
                                tp, wn[i][:, j * 128 : (j + 1) * 128], ident
                            )
                            if name == "wp":
                                # split to per-head base-0 tiles via DMA
                                wp_st = sb.tile([128, 128], BF16, tag="wpst", name="wpst")
                                nc.vector.tensor_copy(out=wp_st, in_=tp)
                                for hh in range(2):
                                    h = 2 * j + hh
                                    nc.sync.dma_start(
                                        out=wpTh[h][:, i * 128 : (i + 1) * 128],
                                        in_=wp_st[64 * hh : 64 * hh + 64, :],
                                    )
                            else:
                                nc.vector.tensor_copy(
                                    out=wT_dst[name][j][:, i * 128 : (i + 1) * 128],
                                    in_=tp,
                                )

                # x loads split into column chunks so they spread across DMA
                # queues and the casts overlap the remaining transfers;
                # t-inner so the first projection tile's inputs (needing both
                # channel halves) are ready earliest
                xf = [
                    sb.tile([128, HW], F32, tag=f"xf{t}", name=f"xf{t}")
                    for t in range(2)
                ]
                for b4 in range(4):
                    for t in range(2):
                        sl = slice(b4 * 1024, (b4 + 1) * 1024)
                        nc.sync.dma_start(out=xf[t][:, sl], in_=x_t[t][:, sl])
                        if t == 0:
                            nc.scalar.copy(out=x16[t][:, sl], in_=xf[t][:, sl])
                        else:
                            nc.vector.tensor_copy(out=x16[t][:, sl], in_=xf[t][:, sl])
                for t in range(2):
                    for b4 in range(2):
                        sl = slice(b4 * 1024, (b4 + 1) * 1024)
                        nc.sync.dma_start(out=xb[t][:, sl], in_=xq_t[t][:, sl])
                        nc.vector.tensor_copy(out=xq16[t][:, sl], in_=xb[t][:, sl])
                    bp_t = sb.tile([128, 1], F32, tag="bpt", name="bpt")
                    nc.sync.dma_start(out=bp_t, in_=bp_r[t])
                    nc.vector.tensor_mul(out=gb[t], in0=bp_t, in1=gam)
                    # xb = xq + gamma*bp
                    nc.vector.tensor_scalar_add(out=xb[t], in0=xb[t], scalar1=gb[t])

            # ---------- K, Q, V projections (128x128 mode) ----------
            with tc.tile_pool(name="proj_ps", bufs=3, space="PSUM") as bps:
                for t in range(2):
                    for mb in range(HW // 512):
                        ps = bps.tile([128, 512], F32, tag="pk", name="pk")
                        for ci in range(2):
                            nc.tensor.matmul(
                                ps,
                                lhsT=wkT[ci][:, 128 * t : 128 * t + 128],
                                rhs=x16[ci][:, mb * 512 : (mb + 1) * 512],
                                start=(ci == 0),
                                stop=(ci == 1),
                            )
                        nc.scalar.activation(
                            out=kp[t][:, mb * 512 : (mb + 1) * 512],
                            in_=ps,
                            func=mybir.ActivationFunctionType.Identity,
                            bias=bkp[t],
                        )
                for t in range(2):
                    for nb in range(NHALF // 512):
                        ps = bps.tile([128, 512], F32, tag="pk", name="pk")
                        for ci in range(2):
                            nc.tensor.matmul(
                                ps,
                                lhsT=wqT[ci][:, 128 * t : 128 * t + 128],
                                rhs=xq16[ci][:, nb * 512 : (nb + 1) * 512],
                                start=(ci == 0),
                                stop=(ci == 1),
                            )
                        nc.scalar.activation(
                            out=qp[t][:, nb * 512 : (nb + 1) * 512],
                            in_=ps,
                            func=mybir.ActivationFunctionType.Identity,
                            bias=bqp[t],
                        )
                for mc in range(NCHUNK):
                    ps = bps.tile([128, C], F32, tag="pv", name="pv")
                    for ci in range(2):
                        nc.tensor.matmul(
                            ps,
                            lhsT=x16[ci][:, mc * 128 : (mc + 1) * 128],
                            rhs=wvT[ci][:, :],
                            start=(ci == 0),
                            stop=(ci == 1),
                        )
                    nc.vector.tensor_add(
                        out=vta[:, mc, :, 0:HD],
                        in0=ps.rearrange("p (h d) -> p h d", h=HEADS),
                        in1=bvb.rearrange("p (h d) -> p h d", h=HEADS),
                    )

            # ---------- attention, entirely in (64,128) tile mode ----------
            with (
                tc.tile_pool(name="st_ps", bufs=2, space="PSUM") as stp,
                tc.tile_pool(name="o_ps", bufs=1, space="PSUM") as op,
                tc.tile_pool(name="attn_sb", bufs=PV_LAG + 2) as asb,
                tc.tile_pool(name="norm_sb", bufs=2) as nsb,
            ):
                def norm_thunks(cA, cB, ha, hb, n0, gidx):
                    """oh[h][:, n0:+QB] = (lo+hi)[0:64] / (lo+hi)[64] from the
                    SBUF-staged [65,1024] drains. Everything runs on GpSimd
                    (add, int-trick reciprocal, scale) and DMA (partition
                    broadcast) so the exp engines stay exp-only."""
                    thunks = []
                    for hi, (cc, h) in enumerate(((cA, ha), (cB, hb))):
                        scr = rscr_d[(gidx % 2) * 2 + hi]
                        tmp = nsb.tile([HD + 1, QB], F32, tag="tmp", name="tmp")
                        rp = nsb.tile([1, QB], mybir.dt.int32, tag="rp", name="rp")
                        bcs = nsb.tile([HD, QB], F32, tag="bcs", name="bcs")

                        def t_add(tmp=tmp, cc=cc):
                            nc.gpsimd.tensor_tensor(
                                out=tmp,
                                in0=cc[:, 0:QB],
                                in1=cc[:, QB : 2 * QB],
                                op=mybir.AluOpType.add,
                            )

                        def t_rp(rp=rp, tmp=tmp):
                            # C - bits(x) as (bits(x) * -1) + C (both arith
                            # ops; bitwise+arith can't mix in one instr)
                            nc.gpsimd.tensor_scalar(
                                out=rp,
                                in0=tmp[HD : HD + 1, :].bitcast(mybir.dt.int32),
                                scalar1=-1,
                                scalar2=RECIP_MAGIC_P1 - 1,
                                op0=mybir.AluOpType.mult,
                                op1=mybir.AluOpType.add,
                            )

                        def t_bc1(rp=rp, scr=scr):
                            # partition broadcast via DRAM bounce: store...
                            nc.sync.dma_start(
                                out=scr[:].rearrange("(one n) -> one n", one=1),
                                in_=rp.bitcast(F32),
                            )

                        def t_bc(bcs=bcs, scr=scr):
                            # ...then reload with a stride-0 partition AP
                            s_ap = scr[:]
                            nc.sync.dma_start(
                                out=bcs,
                                in_=bass.AP(
                                    tensor=s_ap.tensor,
                                    offset=s_ap.offset,
                                    ap=[[0, HD], [1, QB]],
                                ),
                            )

                        def t_mul(h=h, tmp=tmp, bcs=bcs):
                            nc.gpsimd.tensor_tensor(
                                out=oh[h][:, n0 : n0 + QB],
                                in0=tmp[0:HD, :],
                                in1=bcs,
                                op=mybir.AluOpType.mult,
                            )

                        thunks += [t_add, t_rp, t_bc1, t_bc, t_mul]
                    return thunks

                pending = []
                for t in range(2):
                    ha, hb = 2 * t, 2 * t + 1
                    for nb in range(NQB):
                        n0 = nb * QB
                        # per head: one [65,1024] tile spanning two PSUM banks
                        # (lo-keys half in cols 0:512, hi-keys in 512:1024) so
                        # ONE copy per head drains the whole accumulator
                        oA = op.tile([HD + 1, 2 * QB], F32, tag="oA", name="oA")
                        oB = op.tile([HD + 1, 2 * QB], F32, tag="oB", name="oB")

                        def emit_pv(exb, mc):
                            # PV: keys split 64/64 across T0/T8
                            first, last = mc == 0, mc == NCHUNK - 1
                            nc.tensor.matmul(
                                oA[:, 0:QB], lhsT=vta[0:64, mc, ha, :],
                                rhs=exb[0:64, 0:512], start=first, stop=last,
                            )
                            nc.tensor.matmul(
                                oA[:, QB : 2 * QB], lhsT=vta[64:128, mc, ha, :],
                                rhs=exb[64:128, 0:512], start=first, stop=last,
                            )
                            nc.tensor.matmul(
                                oB[:, 0:QB], lhsT=vta[0:64, mc, hb, :],
                                rhs=exb[0:64, 512:1024], start=first, stop=last,
                            )
                            nc.tensor.matmul(
                                oB[:, QB : 2 * QB], lhsT=vta[64:128, mc, hb, :],
                                rhs=exb[64:128, 512:1024], start=first, stop=last,
                            )

                        # software pipeline: PV runs PV_LAG chunks behind
                        # S/exp so the in-order PE never waits on an exp —
                        # it executes earlier chunks' PVs instead
                        from collections import deque

                        inflight = deque()
                        for mc in range(NCHUNK):
                            # trickle the previous group's PSUM drains and
                            # normalize chain BEFORE this chunk's exp so the
                            # drains don't queue behind it on ScalarE/DVE
                            if pending:
                                pending.pop(0)()
                            st = stp.tile([128, 1024], F32, tag="st", name="st")
                            # S for both heads concurrently (T0 rows 0-63,
                            # T8 rows 64-127)
                            nc.tensor.matmul(
                                st[:, 0:512],
                                lhsT=kp[t][0:64, mc * 128 : (mc + 1) * 128],
                                rhs=qp[t][0:64, n0 : n0 + QB],
                                start=True,
                                stop=True,
                            )
                            nc.tensor.matmul(
                                st[:, 512:1024],
                                lhsT=kp[t][64:128, mc * 128 : (mc + 1) * 128],
                                rhs=qp[t][64:128, n0 : n0 + QB],
                                start=True,
                                stop=True,
                            )
                            # exp on alternating engines; the group's last
                            # chunk splits across BOTH engines so the serial
                            # boundary chain exp(31)->PV(31)->drain shortens
                            if mc == NCHUNK - 1:
                                exi = asb.tile([128, 1024], I16, tag="exi", name="exi")
                                exb = exi.bitcast(BF16)
                                nc.scalar.activation(
                                    out=exb[:, 0:512],
                                    in_=st[:, 0:512],
                                    func=mybir.ActivationFunctionType.Exp,
                                    scale=float(SCALE),
                                )
                                nc.vector.tensor_scalar(
                                    out=exi[:, 512:1024],
                                    in0=st[:, 512:1024],
                                    scalar1=SCH_A,
                                    scalar2=SCH_B,
                                    op0=mybir.AluOpType.mult,
                                    op1=mybir.AluOpType.add,
                                )
                            elif EXP_ON_ACT[mc]:
                                ex = asb.tile([128, 1024], BF16, tag="ex", name="ex")
                                nc.scalar.activation(
                                    out=ex,
                                    in_=st,
                                    func=mybir.ActivationFunctionType.Exp,
                                    scale=float(SCALE),
                                )
                                exb = ex
                            else:
                                exi = asb.tile([128, 1024], I16, tag="exi", name="exi")
                                nc.vector.tensor_scalar(
                                    out=exi,
                                    in0=st,
                                    scalar1=SCH_A,
                                    scalar2=SCH_B,
                                    op0=mybir.AluOpType.mult,
                                    op1=mybir.AluOpType.add,
                                )
                                exb = exi.bitcast(BF16)
                            inflight.append((exb, mc))
                            if len(inflight) > PV_LAG:
                                emit_pv(*inflight.popleft())
                        while inflight:
                            emit_pv(*inflight.popleft())
                        while pending:
                            pending.pop(0)()
                        # defer the two ops drains into the next group's
                        # first chunks (emitted before its PV(0) so the bank
                        # reuse dependency stays correct)
                        cA = nsb.tile([HD + 1, 2 * QB], F32, tag="cA", name="cA")
                        cB = nsb.tile([HD + 1, 2 * QB], F32, tag="cB", name="cB")
                        drains = [
                            lambda cA=cA, o=oA: nc.scalar.copy(out=cA, in_=o),
                            lambda cB=cB, o=oB: nc.vector.tensor_copy(out=cB, in_=o),
                        ]
                        pending = drains + norm_thunks(
                            cA, cB, ha, hb, n0, t * NQB + nb
                        )
                for th in pending:
                    th()

                # ------ output projection + residual ((64,128) mode) ------
                # emitted inside the attention pools, with PSUM borrowed
                # from the drained ops slots, so there is no pool-close
                # barrier between the last PV group and the first out-proj
                # matmul; nb-major so blocks 0-2 (ready before the last
                # group's normalize flush) keep the PE busy through the tail
                for nb in range(NHALF // 512):
                    for oc in range(2):
                        tag = "oA" if (nb * 2 + oc) % 2 == 0 else "oB"
                        ps = op.tile([128, 512], F32, tag=tag, name="pp")
                        for h in range(HEADS):
                            nc.tensor.matmul(
                                ps,
                                lhsT=wpTh[h][:, oc * 128 : (oc + 1) * 128],
                                rhs=oh[h][:, nb * 512 : (nb + 1) * 512],
                                start=(h == 0),
                                stop=(h == HEADS - 1),
                            )
                        res = nsb.tile([128, 512], F32, tag="res", name="res", bufs=3)
                        nc.vector.scalar_tensor_tensor(
                            out=res,
                            in0=ps,
                            scalar=gam[:, 0:1],
                            in1=xb[oc][:, nb * 512 : (nb + 1) * 512],
                            op0=mybir.AluOpType.mult,
                            op1=mybir.AluOpType.add,
                        )
                        nc.sync.dma_start(
                            out=out_t[oc, :, nb * 512 : (nb + 1) * 512], in_=res
                        )

    if fix:
        _fix_tail_drain(nc)
    return nc


_NC_CACHE = None


def _get_nc():
    global _NC_CACHE
    if _NC_CACHE is None:
        _NC_CACHE = build()
    return _NC_CACHE


def kernel(x, wq, bq, wk, bk, wv, bv, wp, bp, gamma):
    from concourse.bass_utils import run_bass_kernel_spmd

    nc = _get_nc()
    x = np.ascontiguousarray(np.asarray(x, np.float32)).reshape(B, C, HW)
    common = {
        "wq": np.ascontiguousarray(np.asarray(wq, np.float32)),
        "wk": np.ascontiguousarray(np.asarray(wk, np.float32)),
        "wv": np.ascontiguousarray(np.asarray(wv, np.float32)),
        "wp": np.ascontiguousarray(np.asarray(wp, np.float32)),
        "bq": np.ascontiguousarray(np.asarray(bq, np.float32)),
        "bk": np.ascontiguousarray(np.asarray(bk, np.float32)),
        "bv": np.ascontiguousarray(np.asarray(bv, np.float32)),
        "bp": np.ascontiguousarray(np.asarray(bp, np.float32)),
        "gamma": np.ascontiguousarray(np.asarray(gamma, np.float32)),
    }
    in_maps = []
    for core in range(8):
        b, j = core // 2, core % 2
        m = dict(common)
        m["x"] = np.ascontiguousarray(x[b])
        m["xq"] = np.ascontiguousarray(x[b][:, j * NHALF : (j + 1) * NHALF])
        in_maps.append(m)

    res = run_bass_kernel_spmd(nc, in_maps, core_ids=list(range(8)), trace=False)
    out = np.empty((B, C, HW), np.float32)
    for core in range(8):
        b, j = core // 2, core % 2
        out[b][:, j * NHALF : (j + 1) * NHALF] = res.results[core]["out"]
    return out.reshape(B, C, H, W)
